# revision 42
# baseline (speedup 1.0000x reference)
"""Trainium2 Bass kernel for a dense transformer block (nn_Block_7713761264306).

Sharding: 8 cores = 4 batches x 2 query-halves. Each core computes K/V over the
full sequence for its batch, but runs only its 1024 query rows through
attention and the FFN. The query half is selected by rotating the token axis
host-side (exact: no mask, softmax is permutation-invariant over keys).
No collectives.

Device layout: activations are kept feature-on-partition ([D, tokens]) so every
linear layer is a direct PE matmul (lhsT = weights, rhs = activations^T) with
no on-device transposes. LayerNorm is folded into the matmul pipeline: x is
centered in place (xc = x - mu, mu via fp8 ones-matmul stats), and the
per-token 1/sd is applied at each QKV PSUM drain (row-broadcast tile for K/Q,
a transposed per-token column - obtained via a tiny DRAM round trip - for V).
This removes the normalized-x materialization from the critical path entirely;
QKV matmuls run directly on xc while later column blocks are still being
centered. Softmax runs in S^T layout (keys on partitions, queries free);
denominators come from a ones-column appended to V in the PV matmul. LN1's
gamma and the attn pre-projection fold into W_qkv host-side.

The whole kernel is software-pipelined at emission level (engines execute
their streams in order): LN1 stats/centering per 512-column block interleave
with the first head-pair's K/Q/scores; each attention pair's exp stream (the
ACT bottleneck, ~66% of all cycles) overlaps the previous pair's PV/output
normalization and the next pair's K/Q production; V production hides in the
first pair's exp window; the FFN tail pipelines proj -> LN2 -> FFN1 -> FFN2
per 512-query block with n0-leading chain emission.

fp8: QKV/proj/PV/FFN matmuls run as fp8e4m3 DoubleRow (2 K-chunks per
instruction, 0.5 cyc/row). Weights are pre-scaled (x8/x16/x32) out of e4m3's
subnormal range and unscaled via free activation/stt scalar slots; W2
additionally carries an fp8 residual chain (hi+lo) so its quantization error
is negligible (W1's is dropped - its error hides under the fp8 activation
noise floor). exp outputs fp8 directly with a -2 bias shift (uniform factor
cancels in the softmax normalization) centering probs in e4m3 range. Scores
stay bf16. The residual stream (xtq) stays bf16.
"""

import numpy as np
import ml_dtypes

import concourse.bass as bass
import concourse.mybir as mybir
import concourse.tile as tile
from concourse.bass import ts
from concourse.bass_utils import run_bass_kernel_spmd

BF16 = mybir.dt.bfloat16
F32 = mybir.dt.float32
F8 = mybir.dt.float8e4
bf16 = ml_dtypes.bfloat16
f8e4 = ml_dtypes.float8_e4m3
DR = mybir.MatmulPerfMode.DoubleRow

# fp8 weight pre-scales (undone on-device via free scale slots). The raw
# weight sigmas (~1/32, ~1/64) sit in e4m3's subnormal range; scaling up
# recovers full mantissa precision.
W1_SCALE = 16.0
W2_SCALE = 32.0
WQ_SCALE = 8.0       # W_eff / W_proj pre-scale
ESHIFT = -2.0        # exp bias shift: centers softmax numerators in e4m3
W1_COMP = False      # fp8 residual (hi+lo) chain for W1
W2_COMP = True       # fp8 residual (hi+lo) chain for W2

B, T, D, H, HS, FF = 4, 2048, 1024, 16, 64, 4096
P = 128
DC = D // P          # 8 feature chunks
FC = FF // P         # 32 ffn chunks
TKC = T // P         # 16 key/token chunks
Tq = 1024            # queries per core
NT = T // 512        # 4 column blocks over full seq
NQ = Tq // 512       # 2 column blocks over queries
N_CORES = 8
EPS = 1e-5
NK1 = DC if W1_COMP else DC // 2
NK2 = FC if W2_COMP else FC // 2

AF = mybir.ActivationFunctionType
ALU = mybir.AluOpType


def build_nc(cap=True, nzqkv=False, nzproj=False, nzb2=False):
    nc = bass.Bass()
    io = {}
    io["xt"] = nc.dram_tensor("xt", [DC, P, T], F8, kind="ExternalInput")
    io["xtq"] = nc.dram_tensor("xtq", [DC, P, Tq], BF16,
                               kind="ExternalInput")
    io["wq"] = nc.dram_tensor("wq", [DC, P, DC, P], F8, kind="ExternalInput")
    io["wk"] = nc.dram_tensor("wk", [DC, P, DC, P], F8, kind="ExternalInput")
    io["wv"] = nc.dram_tensor("wv", [DC, P, D], F8, kind="ExternalInput")
    io["bqkv"] = nc.dram_tensor("bqkv", [P, 2 * DC], F32, kind="ExternalInput")
    io["bv"] = nc.dram_tensor("bv", [D], BF16, kind="ExternalInput")
    io["wproj"] = nc.dram_tensor("wproj", [DC, P, DC, P], F8,
                                 kind="ExternalInput")
    io["bproj"] = nc.dram_tensor("bproj", [P, DC], F32, kind="ExternalInput")
    io["w1"] = nc.dram_tensor("w1", [FC, P, 2 * NK1, P], F8,
                              kind="ExternalInput")
    io["b1"] = nc.dram_tensor("b1", [P, FC], F32, kind="ExternalInput")
    io["w2"] = nc.dram_tensor("w2", [DC, P, 2 * NK2, P], F8,
                              kind="ExternalInput")
    io["b2"] = nc.dram_tensor("b2", [P, DC], F32, kind="ExternalInput")
    io["g2"] = nc.dram_tensor("g2", [P, DC], F32, kind="ExternalInput")
    io["bln2"] = nc.dram_tensor("bln2", [P, DC], F32, kind="ExternalInput")
    io["out"] = nc.dram_tensor("out", [DC, P, Tq], F32, kind="ExternalOutput")
    io["scr"] = nc.dram_tensor("scr", [NT, 512], BF16, kind="Internal")

    with tile.TileContext(nc) as tc:
        _emit(nc, tc, io, nzqkv, nzproj, nzb2)
    nc.finalize()
    if cap:
        _cap_waits(nc)
    return nc


def _cap_waits(nc, keep_types=()):
    """This toolchain's walrus accepts only one sync-wait command per compute
    instruction; hoist extra waits into preceding same-engine NoOps."""
    cnt = 0
    for fn in nc.m.functions:
        for blk in fn.blocks:
            new = []
            for inst in blk.instructions:
                si = getattr(inst, "sync_info", None)
                if si is not None and len(si.on_wait) > 1 \
                        and type(inst).__name__ not in keep_types:
                    waits = list(si.on_wait)
                    for w in waits[:-1]:
                        cnt += 1
                        nop = mybir.InstNoOp(
                            name=f"{inst.name}-w{cnt}", ins=[], outs=[])
                        nop.engine = inst.engine
                        nop.sync_info = mybir.SyncInfo(on_wait=[w],
                                                       on_update=[])
                        new.append(nop)
                    inst.sync_info = mybir.SyncInfo(
                        on_wait=[waits[-1]], on_update=list(si.on_update))
                new.append(inst)
            blk.instructions = new
    return cnt


def _emit(nc, tc, io, nzqkv=False, nzproj=False, nzb2=False):
    # Pools release in LIFO order: tail-lived pools sit at the stack bottom,
    # attention-lived above them, phase-A-only PSUM pools on top.
    consts = tc.alloc_tile_pool(name="consts", bufs=1)
    prows = tc.alloc_tile_pool(name="prows", bufs=1)
    prc = tc.alloc_tile_pool(name="prc", bufs=2)
    ph = tc.alloc_tile_pool(name="ph", bufs=1)
    pg = tc.alloc_tile_pool(name="pg", bufs=1)
    pr2 = tc.alloc_tile_pool(name="pr2", bufs=1)
    pxq = tc.alloc_tile_pool(name="pxq", bufs=1)
    pwproj = tc.alloc_tile_pool(name="pwproj", bufs=1)

    # ---------------- attention-lived SBUF pools ----------------
    pxT = tc.alloc_tile_pool(name="pxT", bufs=1)
    pwv = tc.alloc_tile_pool(name="pwv", bufs=1)
    prbm = tc.alloc_tile_pool(name="prbm", bufs=1)
    pcol = tc.alloc_tile_pool(name="pcol", bufs=1)
    pKp = tc.alloc_tile_pool(name="pKp", bufs=2)
    pQp = tc.alloc_tile_pool(name="pQp", bufs=2)
    pwkv = tc.alloc_tile_pool(name="pwkv", bufs=3)
    pvaug = tc.alloc_tile_pool(name="pvaug", bufs=1)
    ppt = tc.alloc_tile_pool(name="ppt", bufs=4)

    # PSUM pools for phase A + attention (8 banks exactly; st/bc release
    # after LN1 block 3 frees banks for V, whose release frees po/rbp).
    ppmm = tc.alloc_tile_pool(name="ppmm", bufs=1, space="PSUM")
    pps = tc.alloc_tile_pool(name="pps", bufs=2, space="PSUM")
    pxsq = tc.alloc_tile_pool(name="pxsq", bufs=1)
    ppst = tc.alloc_tile_pool(name="ppst", bufs=2, space="PSUM")
    ppbc = tc.alloc_tile_pool(name="ppbc", bufs=1, space="PSUM")

    # ------------------------- DMAs first (critical path) ----------------
    xT = pxT.tile([P, DC, T], F8, name="xT")
    xt_r = io["xt"].rearrange("c p t -> p c t")
    for kt in range(NT):
        nc.sync.dma_start(out=xT[:, :, ts(kt, 512)],
                          in_=xt_r[:, :, ts(kt, 512)])

    wkj0 = pwkv.tile([P, DC, P], F8, tag="w", name="wkj0")
    nc.sync.dma_start(out=wkj0, in_=io["wk"][0])
    wqj0 = pwkv.tile([P, DC, P], F8, tag="w", name="wqj0")
    nc.sync.dma_start(out=wqj0, in_=io["wq"][0])
    wv_t = pwv.tile([P, DC, D], F8, name="wv_t")
    nc.sync.dma_start(out=wv_t, in_=io["wv"].rearrange("c p d -> p c d"))

    bvB = consts.tile([P, D], BF16)
    nc.sync.dma_start(out=bvB, in_=io["bv"][:].partition_broadcast(P))
    b1_s = consts.tile([P, FC], F32)
    nc.sync.dma_start(out=b1_s, in_=io["b1"][:])
    g2_s = consts.tile([P, DC], F32)
    nc.sync.dma_start(out=g2_s, in_=io["g2"][:])
    bln2_s = consts.tile([P, DC], F32)
    nc.sync.dma_start(out=bln2_s, in_=io["bln2"][:])
    if nzqkv:
        bqkv_s = consts.tile([P, 2 * DC], F32)
        nc.sync.dma_start(out=bqkv_s, in_=io["bqkv"][:])
    if nzproj:
        bproj_s = consts.tile([P, DC], F32)
        nc.sync.dma_start(out=bproj_s, in_=io["bproj"][:])
    if nzb2:
        b2_s = consts.tile([P, DC], F32)
        nc.sync.dma_start(out=b2_s, in_=io["b2"][:])

    # ------------------------- constants -------------------------
    inv_w2_s = consts.tile([P, 1], F32)
    nc.vector.memset(inv_w2_s, 1.0 / W2_SCALE)
    inv8_s = consts.tile([P, 1], F32)
    nc.vector.memset(inv8_s, 1.0 / WQ_SCALE)
    esh_s = consts.tile([P, 1], F32)
    nc.vector.memset(esh_s, ESHIFT)
    invD1 = consts.tile([1, 1], F32)
    nc.vector.memset(invD1, 1.0 / D)
    invDb = consts.tile([P, 1], BF16)
    nc.vector.memset(invDb, 1.0 / D)
    ones1_f8 = consts.tile([P, 1], F8)
    nc.vector.memset(ones1_f8, 1.0)
    onesK1 = consts.tile([1, P], BF16)
    nc.vector.memset(onesK1, 1.0)
    eps_t = consts.tile([1, 1], F32)
    nc.vector.memset(eps_t, EPS)

    rB = prbm.tile([P, T], BF16, tag="rb", name="rB")
    muB = prbm.tile([P, T], BF16, tag="mb", name="muB")
    rcol = pcol.tile([P, TKC], BF16, name="rcol")
    v_aug = pvaug.tile([P, TKC, H * (HS + 1)], F8, name="v_aug")
    v4 = v_aug.rearrange("p i (h e) -> p i h e", e=HS + 1)
    nc.vector.memset(v4[:, :, :, HS:HS + 1], 1.0)
    bv4 = bvB.rearrange("p (c d) -> p c d", d=HS)
    xq_t = pxq.tile([P, DC, Tq], BF16, name="xq_t")
    wpj = pwproj.tile([P, DC, DC, P], F8, name="wpj")
    h_t = ph.tile([P, DC, Tq], BF16, name="h_t")
    g_t = pg.tile([P, DC, Tq], F8, name="g_t")
    och_t = g_t
    r2B = pr2.tile([P, Tq], BF16, tag="rb", name="r2B")
    mur2B = pr2.tile([P, Tq], BF16, tag="mb", name="mur2B")

    def ln1_block(kt, on_act):
        """Squares, stats, row math, broadcasts, and in-place centering for
        one 512-column block; also writes this block's r/8 row to scr."""
        sl = ts(kt, 512)
        xsq = pxsq.tile([P, DC, 512], F8, tag="xs", name=f"xsq{kt}")
        for c in range(DC):
            if on_act and c < 4:
                nc.scalar.square(xsq[:, c, :], xT[:, c, sl])
            elif c % 2 == 0:
                nc.vector.tensor_mul(xsq[:, c, :], xT[:, c, sl],
                                     xT[:, c, sl])
            else:
                nc.gpsimd.tensor_mul(xsq[:, c, :], xT[:, c, sl],
                                     xT[:, c, sl])
        ps_mu = ppst.tile([1, 512], F32, tag="st", name=f"psmu{kt}")
        for c in range(DC):
            nc.tensor.matmul(ps_mu, ones1_f8, xT[:, c, sl],
                             start=(c == 0), stop=(c == DC - 1))
        ps_sq = ppst.tile([1, 512], F32, tag="st", name=f"pssq{kt}")
        for c in range(DC):
            nc.tensor.matmul(ps_sq, ones1_f8, xsq[:, c, :],
                             start=(c == 0), stop=(c == DC - 1))
        mubf = prows.tile([1, 512], BF16, tag="mub", name=f"mubf{kt}")
        nc.scalar.mul(mubf, ps_mu, 1.0 / D)
        musq = prows.tile([1, 512], F32, tag="msq", name=f"musq{kt}")
        nc.vector.tensor_mul(musq, mubf, mubf)
        nc.vector.scalar_tensor_tensor(out=musq, in0=ps_sq, scalar=invD1,
                                       in1=musq, op0=ALU.mult,
                                       op1=ALU.subtract)
        sd = prows.tile([1, 512], F32, tag="mu", name=f"sd{kt}")
        nc.scalar.activation(out=sd, in_=musq, func=AF.Sqrt, bias=eps_t,
                             scale=1.0)
        r_f = prows.tile([1, 512], F32, tag="msq", name=f"r{kt}")
        nc.vector.reciprocal(out=r_f, in_=sd)
        rrow = prows.tile([1, 512], BF16, tag="rr", name=f"rrow{kt}")
        nc.vector.tensor_scalar_mul(rrow, r_f, 1.0 / WQ_SCALE)
        nc.sync.dma_start(out=io["scr"][kt:kt + 1, :], in_=rrow)
        bp1 = ppbc.tile([P, 512], F32, tag="bc", name=f"bp1{kt}")
        nc.tensor.matmul(bp1, onesK1, rrow)
        cpy = nc.scalar.copy if on_act else nc.vector.tensor_copy
        cpy(out=rB[:, sl], in_=bp1)
        bp2 = ppbc.tile([P, 512], F32, tag="bc", name=f"bp2{kt}")
        nc.tensor.matmul(bp2, onesK1, mubf)
        cpy(out=muB[:, sl], in_=bp2)
        # center x in place: xc = x - mu
        for c in range(DC):
            eng = nc.vector if c % 2 == 0 else nc.gpsimd
            eng.tensor_sub(xT[:, c, sl], xT[:, c, sl], muB[:, sl])

    def k_block(hp, wkj, Kp, kt):
        sl = ts(kt, 512)
        psk = ppmm.tile([P, 512], F32, tag="mm", name=f"psk{hp}_{kt}")
        for c in range(DC // 2):
            nc.tensor.matmul(psk, wkj[:, 2 * c:2 * c + 2, :],
                             xT[:, 2 * c:2 * c + 2, sl],
                             start=(c == 0), stop=(c == DC // 2 - 1),
                             perf_mode=DR)
        nc.vector.tensor_mul(Kp[:, sl], psk, rB[:, sl])
        if nzqkv:
            nc.vector.tensor_scalar_add(Kp[:, sl], Kp[:, sl],
                                        bqkv_s[:, DC + hp:DC + hp + 1])

    def q_block(hp, wqj, Qp, kt):
        sl = ts(kt, 512)
        psq = ppmm.tile([P, 512], F32, tag="mm", name=f"psq{hp}_{kt}")
        for c in range(DC // 2):
            nc.tensor.matmul(psq, wqj[:, 2 * c:2 * c + 2, :],
                             xT[:, 2 * c:2 * c + 2, sl],
                             start=(c == 0), stop=(c == DC // 2 - 1),
                             perf_mode=DR)
        nc.vector.tensor_mul(Qp[:, sl], psq, rB[:, sl])
        if nzqkv:
            nc.vector.tensor_scalar_add(Qp[:, sl], Qp[:, sl],
                                        bqkv_s[:, hp:hp + 1])

    def v_chunk(i, ppv):
        ps = [ppv.tile([P, 512], F32, tag="vps", name=f"psv{i}_{n}")
              for n in range(NQ)]
        for c in range(DC // 2):
            for n in range(NQ):
                nc.tensor.matmul(ps[n], xT[:, 2 * c:2 * c + 2, ts(i, P)],
                                 wv_t[:, 2 * c:2 * c + 2, ts(n, 512)],
                                 start=(c == 0), stop=(c == DC // 2 - 1),
                                 perf_mode=DR)
        for n in range(NQ):
            nc.vector.scalar_tensor_tensor(
                out=v4[:, i, n * DC:(n + 1) * DC, 0:HS],
                in0=ps[n].rearrange("p (h d) -> p h d", d=HS),
                scalar=rcol[:, i:i + 1],
                in1=bv4[:, n * DC:(n + 1) * DC, :],
                op0=ALU.mult, op1=ALU.add)

    def scores_exp(hp, kc, pts, Kp, Qp):
        for lo in (0, 1):
            ps = pps.tile([P, Tq], F32, tag="s", name=f"pss{hp}_{kc}_{lo}")
            for n in range(NQ):
                nc.tensor.matmul(ps[:, ts(n, 512)],
                                 Kp[lo * HS:(lo + 1) * HS, ts(kc, P)],
                                 Qp[lo * HS:(lo + 1) * HS, ts(n, 512)])
            nc.scalar.activation(out=pts[lo][:, kc, :], in_=ps, func=AF.Exp,
                                 bias=esh_s, scale=float(1.0 / np.sqrt(HS)))

    def pv_chain(hp, lo, qt, pts, ppo):
        h = 2 * hp + lo
        po = ppo.tile([HS + 1, 512], F32, tag="po", name=f"po{h}_{qt}")
        for kc in range(TKC // 2):
            nc.tensor.matmul(po,
                             v_aug[:, 2 * kc:2 * kc + 2,
                                   h * (HS + 1):(h + 1) * (HS + 1)],
                             pts[lo][:, 2 * kc:2 * kc + 2, ts(qt, 512)],
                             start=(kc == 0), stop=(kc == TKC // 2 - 1),
                             perf_mode=DR)
        rc = prc.tile([1, 512], F32, tag="rc", name=f"rc{h}_{qt}")
        nc.vector.reciprocal(out=rc, in_=po[HS:HS + 1, :])
        rcb = prc.tile([1, 512], BF16, tag="rcb", name=f"rcb{h}_{qt}")
        nc.vector.tensor_copy(out=rcb, in_=rc)
        return po, rcb

    def pv_finish(hp, qt, povs, pprb):
        rbp = pprb.tile([P, 512], F32, tag="rbp", name=f"rbp{hp}_{qt}")
        for lo in (0, 1):
            nc.tensor.matmul(rbp[lo * HS:(lo + 1) * HS, :], onesK1[:, 0:HS],
                             povs[lo][1])
        rbs = prc.tile([P, 512], BF16, tag="rbs", name=f"rbs{hp}_{qt}")
        nc.vector.tensor_copy(out=rbs, in_=rbp)
        for lo in (0, 1):
            nc.vector.tensor_mul(och_t[lo * HS:(lo + 1) * HS, hp,
                                       ts(qt, 512)],
                                 povs[lo][0][0:HS, :],
                                 rbs[lo * HS:(lo + 1) * HS, :])

    # ================= phase A: blocks 0,1 then pipelined ==============
    ln1_block(0, on_act=True)
    ln1_block(1, on_act=True)
    Kp0 = pKp.tile([P, T], BF16, tag="kp", name="kp0")
    Qp0 = pQp.tile([P, Tq], BF16, tag="qp", name="qp0")
    k_block(0, wkj0, Kp0, 0)
    q_block(0, wqj0, Qp0, 0)
    k_block(0, wkj0, Kp0, 1)
    q_block(0, wqj0, Qp0, 1)

    pts0 = [ppt.tile([P, TKC, Tq], F8, tag="pt", name=f"pt0_{lo}")
            for lo in (0, 1)]
    scores_exp(0, 0, pts0, Kp0, Qp0)
    scores_exp(0, 1, pts0, Kp0, Qp0)
    ln1_block(2, on_act=False)
    k_block(0, wkj0, Kp0, 2)
    scores_exp(0, 2, pts0, Kp0, Qp0)
    scores_exp(0, 3, pts0, Kp0, Qp0)
    ln1_block(3, on_act=False)
    k_block(0, wkj0, Kp0, 3)
    ppbc.release()
    ppst.release()
    pxsq.release()
    # transposed r/8 per-token columns for the V drains
    nc.sync.dma_start(out=rcol.rearrange("p (kt j) -> p kt j", j=NT),
                      in_=io["scr"].rearrange("kt (j p) -> p kt j", p=P))
    ppv = tc.alloc_tile_pool(name="ppv", bufs=2, space="PSUM")

    # attention state for the software pipeline
    kq = (Kp0, Qp0)
    kq_next = None
    pts = pts0
    pts_prev = None
    ppo = None
    pprb = None
    povs = {}

    for hp in range(DC):
        Kp, Qp = kq
        if hp + 1 < DC:
            wkj = pwkv.tile([P, DC, P], F8, tag="w", name=f"wkj{hp + 1}")
            wqj = pwkv.tile([P, DC, P], F8, tag="w", name=f"wqj{hp + 1}")
            Kpn = pKp.tile([P, T], BF16, tag="kp", name=f"kp{hp + 1}")
            Qpn = pQp.tile([P, Tq], BF16, tag="qp", name=f"qp{hp + 1}")
        pv0 = 8 if hp == 1 else 0
        for kc in range(TKC):
            if hp > 1:
                if kc == 0:
                    povs[0] = pv_chain(hp - 1, 0, 0, pts_prev, ppo)
                elif kc == 1:
                    povs[1] = pv_chain(hp - 1, 1, 0, pts_prev, ppo)
                elif kc == 2:
                    pv_finish(hp - 1, 0, povs, pprb)
                elif kc == 3:
                    povs[0] = pv_chain(hp - 1, 0, 1, pts_prev, ppo)
                elif kc == 4:
                    povs[1] = pv_chain(hp - 1, 1, 1, pts_prev, ppo)
                elif kc == 5:
                    pv_finish(hp - 1, 1, povs, pprb)
            elif hp == 1:
                if kc < 8:
                    v_chunk(8 + kc, ppv)
                elif kc == 8:
                    ppv.release()
                    ppo = tc.alloc_tile_pool(name="ppo", bufs=2,
                                             space="PSUM")
                    pprb = tc.alloc_tile_pool(name="pprb", bufs=1,
                                              space="PSUM")
                    povs[0] = pv_chain(0, 0, 0, pts_prev, ppo)
                elif kc == 9:
                    povs[1] = pv_chain(0, 1, 0, pts_prev, ppo)
                elif kc == 10:
                    pv_finish(0, 0, povs, pprb)
                elif kc == 11:
                    povs[0] = pv_chain(0, 0, 1, pts_prev, ppo)
                elif kc == 12:
                    povs[1] = pv_chain(0, 1, 1, pts_prev, ppo)
                elif kc == 13:
                    pv_finish(0, 1, povs, pprb)
            elif hp == 0 and kc >= 8:
                v_chunk(kc - 8, ppv)
            if hp + 1 < DC:
                if kc == 0:
                    nc.sync.dma_start(out=wkj, in_=io["wk"][hp + 1])
                    nc.sync.dma_start(out=wqj, in_=io["wq"][hp + 1])
                elif hp == 0:
                    if kc == 10:
                        q_block(1, wqj, Qpn, 0)
                    elif kc == 11:
                        q_block(1, wqj, Qpn, 1)
                    elif kc >= 12:
                        k_block(1, wkj, Kpn, kc - 12)
                elif kc == 6 + pv0:
                    k_block(hp + 1, wkj, Kpn, 0)
                elif kc == 7 + pv0:
                    k_block(hp + 1, wkj, Kpn, 1)
                elif kc == 2:
                    q_block(hp + 1, wqj, Qpn, 0)
                elif kc == 3:
                    q_block(hp + 1, wqj, Qpn, 1)
                elif kc == 4:
                    k_block(hp + 1, wkj, Kpn, 2)
                elif kc == 5:
                    k_block(hp + 1, wkj, Kpn, 3)
            # prefetch tail weights / residual stream during attention
            if kc == 14:
                if hp == 3:
                    nc.sync.dma_start(
                        out=xq_t, in_=io["xtq"].rearrange("c p t -> p c t"))
                elif hp == 5:
                    nc.sync.dma_start(
                        out=wpj,
                        in_=io["wproj"].rearrange("j p c q -> p j c q"))
            scores_exp(hp, kc, pts, Kp, Qp)
            if hp + 1 < DC and kc == 0:
                pts_next = [ppt.tile([P, TKC, Tq], F8, tag="pt",
                                     name=f"pt{hp + 1}_{lo}")
                            for lo in (0, 1)]
        if hp + 1 < DC:
            kq_next = (Kpn, Qpn)
        pts_prev, pts = pts, (pts_next if hp + 1 < DC else None)
        kq = kq_next

    # tail of the attention pipeline: last pair's PV + normalization
    povs[0] = pv_chain(DC - 1, 0, 0, pts_prev, ppo)
    povs[1] = pv_chain(DC - 1, 1, 0, pts_prev, ppo)
    pv_finish(DC - 1, 0, povs, pprb)
    povs[0] = pv_chain(DC - 1, 0, 1, pts_prev, ppo)
    povs[1] = pv_chain(DC - 1, 1, 1, pts_prev, ppo)
    pv_finish(DC - 1, 1, povs, pprb)

    pprb.release()
    ppo.release()
    pps.release()
    ppmm.release()
    ppt.release()
    pvaug.release()
    pwkv.release()
    pQp.release()
    pKp.release()
    pcol.release()
    prbm.release()
    pwv.release()
    pxT.release()

    # ============ tail: proj -> LN2 -> FFN, pipelined per n-block ========
    # cen reuses xq_t's storage (free after that block's proj drains) and
    # hsq reuses g_t's (free once the block's stats matmuls are done).
    pf1 = tc.alloc_tile_pool(name="pf1", bufs=1)
    pw1 = tc.alloc_tile_pool(name="pw1", bufs=1)
    pw2 = tc.alloc_tile_pool(name="pw2", bufs=8)
    f1g = pf1.tile([P, FC, Tq], F8, name="f1g")
    w1_all = pw1.tile([P, FC, 2 * NK1, P], F8, name="w1_all")
    w1r = io["w1"].rearrange("j p c q -> p j c q")
    w2_tiles = []
    for h2 in range(2):
        nc.sync.dma_start(out=w1_all[:, ts(h2, FC // 2)],
                          in_=w1r[:, ts(h2, FC // 2)])
        for jw in range(4 * h2, 4 * h2 + 4):
            w2t = pw2.tile([P, 2 * NK2, P], F8, tag="w2", name=f"w2t{jw}")
            nc.sync.dma_start(out=w2t, in_=io["w2"][jw])
            w2_tiles.append(w2t)
    w1_tiles = [w1_all[:, j] for j in range(FC)]

    ppE = tc.alloc_tile_pool(name="ppE", bufs=2, space="PSUM")
    ppst2 = tc.alloc_tile_pool(name="ppst2", bufs=2, space="PSUM")
    ppbc2 = tc.alloc_tile_pool(name="ppbc2", bufs=2, space="PSUM")
    hsq = g_t
    cen = xq_t

    for n in range(NQ):
        sl = ts(n, 512)
        for j in range(DC):
            psn = ppE.tile([P, 512], F32, tag="mm", name=f"psp{j}_{n}")
            for c in range(DC // 2):
                nc.tensor.matmul(psn, wpj[:, j, 2 * c:2 * c + 2, :],
                                 och_t[:, 2 * c:2 * c + 2, sl],
                                 start=(c == 0), stop=(c == DC // 2 - 1),
                                 perf_mode=DR)
            if j % 2 == 0:
                nc.vector.scalar_tensor_tensor(
                    out=h_t[:, j, sl], in0=psn, scalar=inv8_s,
                    in1=xq_t[:, j, sl], op0=ALU.mult, op1=ALU.add)
            else:
                nc.scalar.mul(h_t[:, j, sl], psn, 1.0 / WQ_SCALE)
                nc.gpsimd.tensor_add(h_t[:, j, sl], h_t[:, j, sl],
                                     xq_t[:, j, sl])
            if nzproj:
                nc.vector.tensor_scalar_add(h_t[:, j, sl], h_t[:, j, sl],
                                            bproj_s[:, j:j + 1])
    for n in range(NQ):
        sl = ts(n, 512)
        # ---- LN2 for this query block ----
        for c in range(DC):
            if n == 0:
                nc.scalar.square(hsq[:, c, sl], h_t[:, c, sl])
            elif c % 2 == 0:
                nc.vector.tensor_mul(hsq[:, c, sl], h_t[:, c, sl],
                                     h_t[:, c, sl])
            else:
                nc.gpsimd.tensor_mul(hsq[:, c, sl], h_t[:, c, sl],
                                     h_t[:, c, sl])
        ps_mu = ppst2.tile([1, 512], F32, tag="st", name=f"ps2mu{n}")
        for c in range(DC):
            nc.tensor.matmul(ps_mu, invDb, h_t[:, c, sl],
                             start=(c == 0), stop=(c == DC - 1))
        ps_sq = ppst2.tile([1, 512], F32, tag="st", name=f"ps2sq{n}")
        for c in range(DC):
            nc.tensor.matmul(ps_sq, ones1_f8, hsq[:, c, sl],
                             start=(c == 0), stop=(c == DC - 1))
        mu2 = prows.tile([1, 512], F32, tag="mu2", name=f"mu2_{n}")
        nc.scalar.copy(out=mu2, in_=ps_mu)
        musq = prows.tile([1, 512], F32, tag="msq", name=f"musq2{n}")
        nc.vector.tensor_mul(musq, mu2, mu2)
        nc.vector.scalar_tensor_tensor(out=musq, in0=ps_sq, scalar=invD1,
                                       in1=musq, op0=ALU.mult,
                                       op1=ALU.subtract)
        sd = prows.tile([1, 512], F32, tag="mu", name=f"sd2{n}")
        nc.scalar.activation(out=sd, in_=musq, func=AF.Sqrt, bias=eps_t,
                             scale=1.0)
        r_f = prows.tile([1, 512], F32, tag="msq", name=f"r2{n}")
        nc.vector.reciprocal(out=r_f, in_=sd)
        rrow = prows.tile([1, 512], BF16, tag="rr", name=f"r2row{n}")
        nc.scalar.copy(out=rrow, in_=r_f)
        rmurow = prows.tile([1, 512], BF16, tag="rmr", name=f"rmu2row{n}")
        nc.gpsimd.tensor_mul(rmurow, mu2, rrow)
        bp1 = ppbc2.tile([P, 512], F32, tag="bc", name=f"b2p1{n}")
        nc.tensor.matmul(bp1, onesK1, rrow)
        nc.scalar.copy(out=r2B[:, sl], in_=bp1)
        bp2 = ppbc2.tile([P, 512], F32, tag="bc", name=f"b2p2{n}")
        nc.tensor.matmul(bp2, onesK1, rmurow)
        nc.scalar.copy(out=mur2B[:, sl], in_=bp2)
        for c in range(DC):
            eng = nc.gpsimd if c % 4 == 1 else nc.vector
            eng.tensor_mul(cen[:, c, sl], h_t[:, c, sl], r2B[:, sl])
            eng.tensor_sub(cen[:, c, sl], cen[:, c, sl], mur2B[:, sl])
            nc.scalar.activation(out=g_t[:, c, sl], in_=cen[:, c, sl],
                                 func=AF.Gelu, bias=bln2_s[:, c:c + 1],
                                 scale=g2_s[:, c:c + 1])

    ppbc2.release()
    ppst2.release()
    ppE.release()

    # ============ FFN1 + lagged early FFN2 chains for 2 of the j's =====
    NJE = 2
    NKK = FC // 2
    poutc = tc.alloc_tile_pool(name="poutc", bufs=3)
    ppG = tc.alloc_tile_pool(name="ppG", bufs=3, space="PSUM")
    ppGe = tc.alloc_tile_pool(name="ppGe", bufs=NJE, space="PSUM")

    psf = {}

    def f1_chain(j, n):
        if n == 0:
            psf[j] = ppG.tile([P, Tq], F32, tag="mm", name=f"psf{j}")
        for k in range(NK1):
            kr = k % (DC // 2)
            nc.tensor.matmul(psf[j][:, ts(n, 512)],
                             w1_tiles[j][:, 2 * k:2 * k + 2, :],
                             g_t[:, 2 * kr:2 * kr + 2, ts(n, 512)],
                             start=(k == 0), stop=(k == NK1 - 1),
                             perf_mode=DR)
        if n == 1:
            nc.scalar.activation(out=f1g[:, j, :], in_=psf.pop(j),
                                 func=AF.Gelu, bias=b1_s[:, j:j + 1],
                                 scale=1.0 / W1_SCALE)

    def pso_step(psj, j, k, n, start, stop):
        kr = k % (FC // 2)
        nc.tensor.matmul(psj, w2_tiles[j][:, 2 * k:2 * k + 2, :],
                         f1g[:, 2 * kr:2 * kr + 2, ts(n, 512)],
                         start=start, stop=stop, perf_mode=DR)

    def outc_store(psj, j, n):
        outc = poutc.tile([P, 512], F32, tag="oc", name=f"outc{j}_{n}")
        nc.vector.scalar_tensor_tensor(
            out=outc, in0=psj, scalar=inv_w2_s,
            in1=h_t[:, j, ts(n, 512)], op0=ALU.mult, op1=ALU.add)
        if nzb2:
            nc.vector.tensor_scalar_add(outc, outc, b2_s[:, j:j + 1])
        nc.sync.dma_start(out=io["out"][j][:, ts(n, 512)], in_=outc)

    psoE = [ppGe.tile([P, 512], F32, tag="me", name=f"psoE{je}")
            for je in range(NJE)]
    nxt = [0]

    def emit_steps(limit_kk):
        while nxt[0] < min(limit_kk, NKK):
            kk = nxt[0]
            last = kk == NKK - 1
            for je in range(NJE):
                pso_step(psoE[je], je, kk, 0, start=(kk == 0),
                         stop=(last and not W2_COMP))
            if W2_COMP:
                for je in range(NJE):
                    pso_step(psoE[je], je, NKK + kk, 0, start=False,
                             stop=last)
            nxt[0] += 1

    LAG = 6
    LEAD = 2
    for j in range(LEAD):
        f1_chain(j, 0)
    for j in range(LEAD, FC + LEAD):
        if j < FC:
            f1_chain(j, 0)
        f1_chain(j - LEAD, 1)
        emit_steps((j - LEAD - LAG) // 2)
    emit_steps(NKK)
    for je in range(NJE):
        outc_store(psoE[je], je, 0)
    ppGe.release()
    ppG.release()

    # ============ FFN2 remainder + residual + store ============
    ppG2 = tc.alloc_tile_pool(name="ppG2", bufs=4, space="PSUM")
    for n in range(NQ):
        for j in range(DC):
            if n == 0 and j < NJE:
                continue
            psj = ppG2.tile([P, 512], F32, tag="mo", name=f"pso{j}_{n}")
            for k in range(NK2):
                pso_step(psj, j, k, n, start=(k == 0), stop=(k == NK2 - 1))
            outc_store(psj, j, n)

    ppG2.release()
    poutc.release()
    pw2.release()
    pw1.release()
    pf1.release()
    pwproj.release()
    pxq.release()
    pr2.release()
    pg.release()
    ph.release()
    prc.release()
    prows.release()
    consts.release()


# ----------------------------------------------------------------------------
# host side
# ----------------------------------------------------------------------------

def _stripe(v):
    """[n*P] -> [P, n] per-partition striping (feature f = c*P + p)."""
    v = np.asarray(v, np.float32)
    return np.ascontiguousarray(v.reshape(-1, P).T)


def _lhsT_stream8(W, scale):
    """fp8e4m3 lhsT stream of W*scale (scale undone on-device)."""
    din, dout = W.shape
    r = (W * scale).astype(f8e4).reshape(din // P, P, dout // P, P)
    return np.ascontiguousarray(r.transpose(2, 1, 0, 3))


def _lhsT_stream8c(W, scale):
    """Compensated fp8 lhsT stream: [Dout/P, P, 2*Din/P, P] with hi rounds
    first, then the fp8-quantized residuals at the same scale (residuals land
    in e4m3's subnormal range whose fixed absolute step makes W-quant error
    negligible). Device accumulates both chains in one PSUM group."""
    din, dout = W.shape
    r = (np.asarray(W, np.float32) * scale)
    hi = r.astype(f8e4)
    lo = (r - hi.astype(np.float32)).astype(f8e4)
    full = np.concatenate([hi.reshape(din // P, P, dout // P, P),
                           lo.reshape(din // P, P, dout // P, P)], axis=0)
    return np.ascontiguousarray(full.transpose(2, 1, 0, 3))


def prep_shared(inputs):
    f32 = np.float32
    g1 = np.asarray(inputs["ln1_g"], f32)
    b1n = np.asarray(inputs["ln1_b"], f32)
    W_ap = np.asarray(inputs["W_ap"], f32)
    b_ap = np.asarray(inputs["b_ap"], f32)
    W_qkv = np.asarray(inputs["W_qkv"], f32)
    b_qkv = np.asarray(inputs["b_qkv"], f32)
    W_proj = np.asarray(inputs["W_proj"], f32)

    # fold LN1 gamma and the whole attn pre-projection into W_qkv:
    # qkv = ln1(x) @ W_ap' @ W_qkv + (b_ap' @ W_qkv + b_qkv)
    W_eff = (g1[:, None] * W_ap) @ W_qkv
    b_eff = (b_ap + b1n @ W_ap) @ W_qkv + b_qkv
    shared = {
        "wq": _lhsT_stream8(W_eff[:, 0:D], WQ_SCALE),
        "wk": _lhsT_stream8(W_eff[:, D:2 * D], WQ_SCALE),
        "wv": np.ascontiguousarray(
            (W_eff[:, 2 * D:] * WQ_SCALE).astype(f8e4).reshape(DC, P, D)),
        "bqkv": _stripe(b_eff[:2 * D]),
        "bv": np.ascontiguousarray(np.asarray(b_eff[2 * D:], f32)),
        "wproj": _lhsT_stream8(W_proj, WQ_SCALE),
        "bproj": _stripe(np.asarray(inputs["b_proj"], f32)),
        "w1": (_lhsT_stream8c if W1_COMP else _lhsT_stream8)(
            np.asarray(inputs["W1"], f32), W1_SCALE),
        "b1": _stripe(np.asarray(inputs["b1"], f32)),
        "w2": (_lhsT_stream8c if W2_COMP else _lhsT_stream8)(
            np.asarray(inputs["W2"], f32), W2_SCALE),
        "b2": _stripe(np.asarray(inputs["b2"], f32)),
        "g2": _stripe(np.asarray(inputs["ln2_g"], f32)),
        "bln2": _stripe(np.asarray(inputs["ln2_b"], f32)),
    }
    return shared


def nz_flags(inputs):
    f32 = np.float32
    g1 = np.asarray(inputs["ln1_g"], f32)
    b1n = np.asarray(inputs["ln1_b"], f32)
    W_ap = np.asarray(inputs["W_ap"], f32)
    b_ap = np.asarray(inputs["b_ap"], f32)
    W_qkv = np.asarray(inputs["W_qkv"], f32)
    b_eff = (b_ap + b1n @ W_ap) @ W_qkv + np.asarray(inputs["b_qkv"], f32)
    return dict(
        nzqkv=bool(np.any(b_eff[:2 * D])),
        nzproj=bool(np.any(np.asarray(inputs["b_proj"]))),
        nzb2=bool(np.any(np.asarray(inputs["b2"]))),
    )


def prep_core_x(x, core):
    b, qh = core // 2, core % 2
    xTb = np.asarray(x[b], np.float32).T  # [D, T] view
    if qh:
        xTb = np.concatenate([xTb[:, Tq:], xTb[:, :Tq]], axis=1)
    return {
        "xt": np.ascontiguousarray(xTb.astype(f8e4).reshape(DC, P, T)),
        "xtq": np.ascontiguousarray(
            xTb[:, :Tq].astype(bf16).reshape(DC, P, Tq)),
    }


def assemble_output(results, dtype):
    out = np.empty((B, T, D), dtype)
    for c in range(N_CORES):
        b, qh = c // 2, c % 2
        arr = np.asarray(results[c]["out"]).reshape(D, Tq)
        out[b, qh * Tq:(qh + 1) * Tq, :] = arr.T
    return out


def kernel(**inputs):
    x = np.asarray(inputs["x"], np.float32)
    shared = prep_shared(inputs)
    nc = build_nc(**nz_flags(inputs))
    in_maps = [dict(shared, **prep_core_x(x, c)) for c in range(N_CORES)]
    res = run_bass_kernel_spmd(nc, in_maps, list(range(N_CORES)))
    return assemble_output(res.results, np.float32)


if __name__ == "__main__":
    nc = build_nc()
    print("built ok")


# revision 43
# speedup vs baseline: 1.0037x; 1.0037x over previous
"""Trainium2 Bass kernel for a dense transformer block (nn_Block_7713761264306).

Sharding: 8 cores = 4 batches x 2 query-halves. Each core computes K/V over the
full sequence for its batch, but runs only its 1024 query rows through
attention and the FFN. The query half is selected by rotating the token axis
host-side (exact: no mask, softmax is permutation-invariant over keys).
No collectives.

Device layout: activations are kept feature-on-partition ([D, tokens]) so every
linear layer is a direct PE matmul (lhsT = weights, rhs = activations^T) with
no on-device transposes. LayerNorm is folded into the matmul pipeline: x is
centered in place (xc = x - mu, mu via fp8 ones-matmul stats), and the
per-token 1/sd is applied at each QKV PSUM drain (row-broadcast tile for K/Q,
a transposed per-token column - obtained via a tiny DRAM round trip - for V).
This removes the normalized-x materialization from the critical path entirely;
QKV matmuls run directly on xc while later column blocks are still being
centered. Softmax runs in S^T layout (keys on partitions, queries free);
denominators come from a ones-column appended to V in the PV matmul. LN1's
gamma and the attn pre-projection fold into W_qkv host-side.

The whole kernel is software-pipelined at emission level (engines execute
their streams in order): LN1 stats/centering per 512-column block interleave
with the first head-pair's K/Q/scores; each attention pair's exp stream (the
ACT bottleneck, ~66% of all cycles) overlaps the previous pair's PV/output
normalization and the next pair's K/Q production; V production hides in the
first pair's exp window; the FFN tail pipelines proj -> LN2 -> FFN1 -> FFN2
per 512-query block with n0-leading chain emission.

fp8: QKV/proj/PV/FFN matmuls run as fp8e4m3 DoubleRow (2 K-chunks per
instruction, 0.5 cyc/row). Weights are pre-scaled (x8/x16/x32) out of e4m3's
subnormal range and unscaled via free activation/stt scalar slots; W2
additionally carries an fp8 residual chain (hi+lo) so its quantization error
is negligible (W1's is dropped - its error hides under the fp8 activation
noise floor). exp outputs fp8 directly with a -2 bias shift (uniform factor
cancels in the softmax normalization) centering probs in e4m3 range. Scores
stay bf16. The residual stream (xtq) stays bf16.
"""

import numpy as np
import ml_dtypes

import concourse.bass as bass
import concourse.mybir as mybir
import concourse.tile as tile
from concourse.bass import ts
from concourse.bass_utils import run_bass_kernel_spmd

BF16 = mybir.dt.bfloat16
F32 = mybir.dt.float32
F8 = mybir.dt.float8e4
bf16 = ml_dtypes.bfloat16
f8e4 = ml_dtypes.float8_e4m3
DR = mybir.MatmulPerfMode.DoubleRow

# fp8 weight pre-scales (undone on-device via free scale slots). The raw
# weight sigmas (~1/32, ~1/64) sit in e4m3's subnormal range; scaling up
# recovers full mantissa precision.
W1_SCALE = 16.0
W2_SCALE = 32.0
WQ_SCALE = 8.0       # W_eff / W_proj pre-scale
ESHIFT = -2.0        # exp bias shift: centers softmax numerators in e4m3
W1_COMP = False      # fp8 residual (hi+lo) chain for W1
W2_COMP = True       # fp8 residual (hi+lo) chain for W2

B, T, D, H, HS, FF = 4, 2048, 1024, 16, 64, 4096
P = 128
DC = D // P          # 8 feature chunks
FC = FF // P         # 32 ffn chunks
TKC = T // P         # 16 key/token chunks
Tq = 1024            # queries per core
NT = T // 512        # 4 column blocks over full seq
NQ = Tq // 512       # 2 column blocks over queries
N_CORES = 8
EPS = 1e-5
NK1 = DC if W1_COMP else DC // 2
NK2 = FC if W2_COMP else FC // 2

AF = mybir.ActivationFunctionType
ALU = mybir.AluOpType


def build_nc(cap=True, nzqkv=False, nzproj=False, nzb2=False):
    nc = bass.Bass()
    io = {}
    io["xt"] = nc.dram_tensor("xt", [DC, P, T], F8, kind="ExternalInput")
    io["xtq"] = nc.dram_tensor("xtq", [DC, P, Tq], BF16,
                               kind="ExternalInput")
    io["wq"] = nc.dram_tensor("wq", [DC, P, DC, P], F8, kind="ExternalInput")
    io["wk"] = nc.dram_tensor("wk", [DC, P, DC, P], F8, kind="ExternalInput")
    io["wv"] = nc.dram_tensor("wv", [DC, P, D], F8, kind="ExternalInput")
    io["bqkv"] = nc.dram_tensor("bqkv", [P, 2 * DC], F32, kind="ExternalInput")
    io["bv"] = nc.dram_tensor("bv", [D], BF16, kind="ExternalInput")
    io["wproj"] = nc.dram_tensor("wproj", [DC, P, DC, P], F8,
                                 kind="ExternalInput")
    io["bproj"] = nc.dram_tensor("bproj", [P, DC], F32, kind="ExternalInput")
    io["w1"] = nc.dram_tensor("w1", [FC, P, 2 * NK1, P], F8,
                              kind="ExternalInput")
    io["b1"] = nc.dram_tensor("b1", [P, FC], F32, kind="ExternalInput")
    io["w2"] = nc.dram_tensor("w2", [DC, P, 2 * NK2, P], F8,
                              kind="ExternalInput")
    io["b2"] = nc.dram_tensor("b2", [P, DC], F32, kind="ExternalInput")
    io["g2"] = nc.dram_tensor("g2", [P, DC], F32, kind="ExternalInput")
    io["bln2"] = nc.dram_tensor("bln2", [P, DC], F32, kind="ExternalInput")
    io["out"] = nc.dram_tensor("out", [DC, P, Tq], F32, kind="ExternalOutput")
    io["scr"] = nc.dram_tensor("scr", [NT, 512], BF16, kind="Internal")

    with tile.TileContext(nc) as tc:
        _emit(nc, tc, io, nzqkv, nzproj, nzb2)
    nc.finalize()
    if cap:
        _cap_waits(nc)
    return nc


def _cap_waits(nc, keep_types=()):
    """This toolchain's walrus accepts only one sync-wait command per compute
    instruction; hoist extra waits into preceding same-engine NoOps."""
    cnt = 0
    for fn in nc.m.functions:
        for blk in fn.blocks:
            new = []
            for inst in blk.instructions:
                si = getattr(inst, "sync_info", None)
                if si is not None and len(si.on_wait) > 1 \
                        and type(inst).__name__ not in keep_types:
                    waits = list(si.on_wait)
                    for w in waits[:-1]:
                        cnt += 1
                        nop = mybir.InstNoOp(
                            name=f"{inst.name}-w{cnt}", ins=[], outs=[])
                        nop.engine = inst.engine
                        nop.sync_info = mybir.SyncInfo(on_wait=[w],
                                                       on_update=[])
                        new.append(nop)
                    inst.sync_info = mybir.SyncInfo(
                        on_wait=[waits[-1]], on_update=list(si.on_update))
                new.append(inst)
            blk.instructions = new
    return cnt


def _emit(nc, tc, io, nzqkv=False, nzproj=False, nzb2=False):
    # Pools release in LIFO order: tail-lived pools sit at the stack bottom,
    # attention-lived above them, phase-A-only PSUM pools on top.
    consts = tc.alloc_tile_pool(name="consts", bufs=1)
    prows = tc.alloc_tile_pool(name="prows", bufs=1)
    prc = tc.alloc_tile_pool(name="prc", bufs=2)
    ph = tc.alloc_tile_pool(name="ph", bufs=1)
    pg = tc.alloc_tile_pool(name="pg", bufs=1)
    pr2 = tc.alloc_tile_pool(name="pr2", bufs=1)
    pxq = tc.alloc_tile_pool(name="pxq", bufs=1)
    pwproj = tc.alloc_tile_pool(name="pwproj", bufs=1)

    # ---------------- attention-lived SBUF pools ----------------
    pxT = tc.alloc_tile_pool(name="pxT", bufs=1)
    pwv = tc.alloc_tile_pool(name="pwv", bufs=1)
    prbm = tc.alloc_tile_pool(name="prbm", bufs=1)
    pcol = tc.alloc_tile_pool(name="pcol", bufs=1)
    pKp = tc.alloc_tile_pool(name="pKp", bufs=2)
    pQp = tc.alloc_tile_pool(name="pQp", bufs=2)
    pwkv = tc.alloc_tile_pool(name="pwkv", bufs=3)
    pvaug = tc.alloc_tile_pool(name="pvaug", bufs=1)
    ppt = tc.alloc_tile_pool(name="ppt", bufs=4)

    # PSUM pools for phase A + attention (8 banks exactly; st/bc release
    # after LN1 block 3 frees banks for V, whose release frees po/rbp).
    ppmm = tc.alloc_tile_pool(name="ppmm", bufs=1, space="PSUM")
    pps = tc.alloc_tile_pool(name="pps", bufs=2, space="PSUM")
    pxsq = tc.alloc_tile_pool(name="pxsq", bufs=1)
    ppst = tc.alloc_tile_pool(name="ppst", bufs=2, space="PSUM")
    ppbc = tc.alloc_tile_pool(name="ppbc", bufs=1, space="PSUM")

    # ------------------------- DMAs first (critical path) ----------------
    xT = pxT.tile([P, DC, T], F8, name="xT")
    xt_r = io["xt"].rearrange("c p t -> p c t")
    for kt in range(NT):
        nc.sync.dma_start(out=xT[:, :, ts(kt, 512)],
                          in_=xt_r[:, :, ts(kt, 512)])

    wkj0 = pwkv.tile([P, DC, P], F8, tag="w", name="wkj0")
    nc.sync.dma_start(out=wkj0, in_=io["wk"][0])
    wqj0 = pwkv.tile([P, DC, P], F8, tag="w", name="wqj0")
    nc.sync.dma_start(out=wqj0, in_=io["wq"][0])
    wv_t = pwv.tile([P, DC, D], F8, name="wv_t")
    nc.sync.dma_start(out=wv_t, in_=io["wv"].rearrange("c p d -> p c d"))

    bvB = consts.tile([P, D], BF16)
    nc.sync.dma_start(out=bvB, in_=io["bv"][:].partition_broadcast(P))
    b1_s = consts.tile([P, FC], F32)
    nc.sync.dma_start(out=b1_s, in_=io["b1"][:])
    g2_s = consts.tile([P, DC], F32)
    nc.sync.dma_start(out=g2_s, in_=io["g2"][:])
    bln2_s = consts.tile([P, DC], F32)
    nc.sync.dma_start(out=bln2_s, in_=io["bln2"][:])
    if nzqkv:
        bqkv_s = consts.tile([P, 2 * DC], F32)
        nc.sync.dma_start(out=bqkv_s, in_=io["bqkv"][:])
    if nzproj:
        bproj_s = consts.tile([P, DC], F32)
        nc.sync.dma_start(out=bproj_s, in_=io["bproj"][:])
    if nzb2:
        b2_s = consts.tile([P, DC], F32)
        nc.sync.dma_start(out=b2_s, in_=io["b2"][:])

    # ------------------------- constants -------------------------
    inv_w2_s = consts.tile([P, 1], F32)
    nc.vector.memset(inv_w2_s, 1.0 / W2_SCALE)
    inv8_s = consts.tile([P, 1], F32)
    nc.vector.memset(inv8_s, 1.0 / WQ_SCALE)
    esh_s = consts.tile([P, 1], F32)
    nc.vector.memset(esh_s, ESHIFT)
    invD1 = consts.tile([1, 1], F32)
    nc.vector.memset(invD1, 1.0 / D)
    invDb = consts.tile([P, 1], BF16)
    nc.vector.memset(invDb, 1.0 / D)
    ones1_f8 = consts.tile([P, 1], F8)
    nc.vector.memset(ones1_f8, 1.0)
    onesK1 = consts.tile([1, P], BF16)
    nc.vector.memset(onesK1, 1.0)
    eps_t = consts.tile([1, 1], F32)
    nc.vector.memset(eps_t, EPS)

    rB = prbm.tile([P, T], BF16, tag="rb", name="rB")
    muB = prbm.tile([P, T], BF16, tag="mb", name="muB")
    rcol = pcol.tile([P, TKC], BF16, name="rcol")
    v_aug = pvaug.tile([P, TKC, H * (HS + 1)], F8, name="v_aug")
    v4 = v_aug.rearrange("p i (h e) -> p i h e", e=HS + 1)
    nc.vector.memset(v4[:, :, :, HS:HS + 1], 1.0)
    bv4 = bvB.rearrange("p (c d) -> p c d", d=HS)
    xq_t = pxq.tile([P, DC, Tq], BF16, name="xq_t")
    wpj = pwproj.tile([P, DC, DC, P], F8, name="wpj")
    h_t = ph.tile([P, DC, Tq], BF16, name="h_t")
    g_t = pg.tile([P, DC, Tq], F8, name="g_t")
    och_t = g_t
    r2B = pr2.tile([P, Tq], BF16, tag="rb", name="r2B")
    mur2B = pr2.tile([P, Tq], BF16, tag="mb", name="mur2B")

    def ln1_block(kt, on_act):
        """Squares, stats, row math, broadcasts, and in-place centering for
        one 512-column block; also writes this block's r/8 row to scr."""
        sl = ts(kt, 512)
        xsq = pxsq.tile([P, DC, 512], F8, tag="xs", name=f"xsq{kt}")
        for c in range(DC):
            if on_act and c < 4:
                nc.scalar.square(xsq[:, c, :], xT[:, c, sl])
            elif c % 2 == 0:
                nc.vector.tensor_mul(xsq[:, c, :], xT[:, c, sl],
                                     xT[:, c, sl])
            else:
                nc.gpsimd.tensor_mul(xsq[:, c, :], xT[:, c, sl],
                                     xT[:, c, sl])
        ps_mu = ppst.tile([1, 512], F32, tag="st", name=f"psmu{kt}")
        for c in range(DC):
            nc.tensor.matmul(ps_mu, ones1_f8, xT[:, c, sl],
                             start=(c == 0), stop=(c == DC - 1))
        ps_sq = ppst.tile([1, 512], F32, tag="st", name=f"pssq{kt}")
        for c in range(DC):
            nc.tensor.matmul(ps_sq, ones1_f8, xsq[:, c, :],
                             start=(c == 0), stop=(c == DC - 1))
        mu_f = prows.tile([1, 512], F32, tag="mu", name=f"mu{kt}")
        nc.vector.tensor_scalar_mul(mu_f, ps_mu, 1.0 / D)
        mubf = prows.tile([1, 512], BF16, tag="mub", name=f"mubf{kt}")
        nc.scalar.copy(out=mubf, in_=mu_f)
        musq = prows.tile([1, 512], F32, tag="msq", name=f"musq{kt}")
        nc.vector.tensor_mul(musq, mu_f, mu_f)
        nc.vector.scalar_tensor_tensor(out=musq, in0=ps_sq, scalar=invD1,
                                       in1=musq, op0=ALU.mult,
                                       op1=ALU.subtract)
        sd = prows.tile([1, 512], F32, tag="mu", name=f"sd{kt}")
        nc.scalar.activation(out=sd, in_=musq, func=AF.Sqrt, bias=eps_t,
                             scale=1.0)
        r_f = prows.tile([1, 512], F32, tag="msq", name=f"r{kt}")
        nc.vector.reciprocal(out=r_f, in_=sd)
        rrow = prows.tile([1, 512], BF16, tag="rr", name=f"rrow{kt}")
        nc.vector.tensor_scalar_mul(rrow, r_f, 1.0 / WQ_SCALE)
        nc.sync.dma_start(out=io["scr"][kt:kt + 1, :], in_=rrow)
        bp1 = ppbc.tile([P, 512], F32, tag="bc", name=f"bp1{kt}")
        nc.tensor.matmul(bp1, onesK1, rrow)
        cpy = nc.scalar.copy if on_act else nc.vector.tensor_copy
        cpy(out=rB[:, sl], in_=bp1)
        bp2 = ppbc.tile([P, 512], F32, tag="bc", name=f"bp2{kt}")
        nc.tensor.matmul(bp2, onesK1, mubf)
        cpy(out=muB[:, sl], in_=bp2)
        # center x in place: xc = x - mu
        for c in range(DC):
            eng = nc.vector if c % 2 == 0 else nc.gpsimd
            eng.tensor_sub(xT[:, c, sl], xT[:, c, sl], muB[:, sl])

    def k_block(hp, wkj, Kp, kt):
        sl = ts(kt, 512)
        psk = ppmm.tile([P, 512], F32, tag="mm", name=f"psk{hp}_{kt}")
        for c in range(DC // 2):
            nc.tensor.matmul(psk, wkj[:, 2 * c:2 * c + 2, :],
                             xT[:, 2 * c:2 * c + 2, sl],
                             start=(c == 0), stop=(c == DC // 2 - 1),
                             perf_mode=DR)
        nc.vector.tensor_mul(Kp[:, sl], psk, rB[:, sl])
        if nzqkv:
            nc.vector.tensor_scalar_add(Kp[:, sl], Kp[:, sl],
                                        bqkv_s[:, DC + hp:DC + hp + 1])

    def q_block(hp, wqj, Qp, kt):
        sl = ts(kt, 512)
        psq = ppmm.tile([P, 512], F32, tag="mm", name=f"psq{hp}_{kt}")
        for c in range(DC // 2):
            nc.tensor.matmul(psq, wqj[:, 2 * c:2 * c + 2, :],
                             xT[:, 2 * c:2 * c + 2, sl],
                             start=(c == 0), stop=(c == DC // 2 - 1),
                             perf_mode=DR)
        nc.vector.tensor_mul(Qp[:, sl], psq, rB[:, sl])
        if nzqkv:
            nc.vector.tensor_scalar_add(Qp[:, sl], Qp[:, sl],
                                        bqkv_s[:, hp:hp + 1])

    def v_chunk(i, ppv):
        ps = [ppv.tile([P, 512], F32, tag="vps", name=f"psv{i}_{n}")
              for n in range(NQ)]
        for c in range(DC // 2):
            for n in range(NQ):
                nc.tensor.matmul(ps[n], xT[:, 2 * c:2 * c + 2, ts(i, P)],
                                 wv_t[:, 2 * c:2 * c + 2, ts(n, 512)],
                                 start=(c == 0), stop=(c == DC // 2 - 1),
                                 perf_mode=DR)
        for n in range(NQ):
            nc.vector.scalar_tensor_tensor(
                out=v4[:, i, n * DC:(n + 1) * DC, 0:HS],
                in0=ps[n].rearrange("p (h d) -> p h d", d=HS),
                scalar=rcol[:, i:i + 1],
                in1=bv4[:, n * DC:(n + 1) * DC, :],
                op0=ALU.mult, op1=ALU.add)

    def scores_exp(hp, kc, pts, Kp, Qp):
        for lo in (0, 1):
            ps = pps.tile([P, Tq], F32, tag="s", name=f"pss{hp}_{kc}_{lo}")
            for n in range(NQ):
                nc.tensor.matmul(ps[:, ts(n, 512)],
                                 Kp[lo * HS:(lo + 1) * HS, ts(kc, P)],
                                 Qp[lo * HS:(lo + 1) * HS, ts(n, 512)])
            nc.scalar.activation(out=pts[lo][:, kc, :], in_=ps, func=AF.Exp,
                                 bias=esh_s, scale=float(1.0 / np.sqrt(HS)))

    def pv_chain(hp, lo, qt, pts, ppo):
        h = 2 * hp + lo
        po = ppo.tile([HS + 1, 512], F32, tag="po", name=f"po{h}_{qt}")
        for kc in range(TKC // 2):
            nc.tensor.matmul(po,
                             v_aug[:, 2 * kc:2 * kc + 2,
                                   h * (HS + 1):(h + 1) * (HS + 1)],
                             pts[lo][:, 2 * kc:2 * kc + 2, ts(qt, 512)],
                             start=(kc == 0), stop=(kc == TKC // 2 - 1),
                             perf_mode=DR)
        rc = prc.tile([1, 512], F32, tag="rc", name=f"rc{h}_{qt}")
        nc.vector.reciprocal(out=rc, in_=po[HS:HS + 1, :])
        rcb = prc.tile([1, 512], BF16, tag="rcb", name=f"rcb{h}_{qt}")
        nc.vector.tensor_copy(out=rcb, in_=rc)
        return po, rcb

    def pv_finish(hp, qt, povs, pprb):
        rbp = pprb.tile([P, 512], F32, tag="rbp", name=f"rbp{hp}_{qt}")
        for lo in (0, 1):
            nc.tensor.matmul(rbp[lo * HS:(lo + 1) * HS, :], onesK1[:, 0:HS],
                             povs[lo][1])
        rbs = prc.tile([P, 512], BF16, tag="rbs", name=f"rbs{hp}_{qt}")
        nc.vector.tensor_copy(out=rbs, in_=rbp)
        for lo in (0, 1):
            nc.vector.tensor_mul(och_t[lo * HS:(lo + 1) * HS, hp,
                                       ts(qt, 512)],
                                 povs[lo][0][0:HS, :],
                                 rbs[lo * HS:(lo + 1) * HS, :])

    # ================= phase A: blocks 0,1 then pipelined ==============
    ln1_block(0, on_act=True)
    ln1_block(1, on_act=True)
    Kp0 = pKp.tile([P, T], BF16, tag="kp", name="kp0")
    Qp0 = pQp.tile([P, Tq], BF16, tag="qp", name="qp0")
    k_block(0, wkj0, Kp0, 0)
    q_block(0, wqj0, Qp0, 0)
    k_block(0, wkj0, Kp0, 1)
    q_block(0, wqj0, Qp0, 1)

    pts0 = [ppt.tile([P, TKC, Tq], F8, tag="pt", name=f"pt0_{lo}")
            for lo in (0, 1)]
    scores_exp(0, 0, pts0, Kp0, Qp0)
    scores_exp(0, 1, pts0, Kp0, Qp0)
    ln1_block(2, on_act=False)
    k_block(0, wkj0, Kp0, 2)
    scores_exp(0, 2, pts0, Kp0, Qp0)
    scores_exp(0, 3, pts0, Kp0, Qp0)
    ln1_block(3, on_act=False)
    k_block(0, wkj0, Kp0, 3)
    ppbc.release()
    ppst.release()
    pxsq.release()
    # transposed r/8 per-token columns for the V drains
    nc.sync.dma_start(out=rcol.rearrange("p (kt j) -> p kt j", j=NT),
                      in_=io["scr"].rearrange("kt (j p) -> p kt j", p=P))
    ppv = tc.alloc_tile_pool(name="ppv", bufs=2, space="PSUM")

    # attention state for the software pipeline
    kq = (Kp0, Qp0)
    kq_next = None
    pts = pts0
    pts_prev = None
    ppo = None
    pprb = None
    povs = {}

    for hp in range(DC):
        Kp, Qp = kq
        if hp + 1 < DC:
            wkj = pwkv.tile([P, DC, P], F8, tag="w", name=f"wkj{hp + 1}")
            wqj = pwkv.tile([P, DC, P], F8, tag="w", name=f"wqj{hp + 1}")
            Kpn = pKp.tile([P, T], BF16, tag="kp", name=f"kp{hp + 1}")
            Qpn = pQp.tile([P, Tq], BF16, tag="qp", name=f"qp{hp + 1}")
        pv0 = 8 if hp == 1 else 0
        for kc in range(TKC):
            if hp > 1:
                if kc == 0:
                    povs[0] = pv_chain(hp - 1, 0, 0, pts_prev, ppo)
                elif kc == 1:
                    povs[1] = pv_chain(hp - 1, 1, 0, pts_prev, ppo)
                elif kc == 2:
                    pv_finish(hp - 1, 0, povs, pprb)
                elif kc == 3:
                    povs[0] = pv_chain(hp - 1, 0, 1, pts_prev, ppo)
                elif kc == 4:
                    povs[1] = pv_chain(hp - 1, 1, 1, pts_prev, ppo)
                elif kc == 5:
                    pv_finish(hp - 1, 1, povs, pprb)
            elif hp == 1:
                if kc < 8:
                    v_chunk(8 + kc, ppv)
                elif kc == 8:
                    ppv.release()
                    ppo = tc.alloc_tile_pool(name="ppo", bufs=2,
                                             space="PSUM")
                    pprb = tc.alloc_tile_pool(name="pprb", bufs=1,
                                              space="PSUM")
                    povs[0] = pv_chain(0, 0, 0, pts_prev, ppo)
                elif kc == 9:
                    povs[1] = pv_chain(0, 1, 0, pts_prev, ppo)
                elif kc == 10:
                    pv_finish(0, 0, povs, pprb)
                elif kc == 11:
                    povs[0] = pv_chain(0, 0, 1, pts_prev, ppo)
                elif kc == 12:
                    povs[1] = pv_chain(0, 1, 1, pts_prev, ppo)
                elif kc == 13:
                    pv_finish(0, 1, povs, pprb)
            elif hp == 0 and kc >= 8:
                v_chunk(kc - 8, ppv)
            if hp + 1 < DC:
                if kc == 0:
                    nc.sync.dma_start(out=wkj, in_=io["wk"][hp + 1])
                    nc.sync.dma_start(out=wqj, in_=io["wq"][hp + 1])
                elif hp == 0:
                    if kc == 10:
                        q_block(1, wqj, Qpn, 0)
                    elif kc == 11:
                        q_block(1, wqj, Qpn, 1)
                    elif kc >= 12:
                        k_block(1, wkj, Kpn, kc - 12)
                elif kc == 6 + pv0:
                    k_block(hp + 1, wkj, Kpn, 0)
                elif kc == 7 + pv0:
                    k_block(hp + 1, wkj, Kpn, 1)
                elif kc == 2:
                    q_block(hp + 1, wqj, Qpn, 0)
                elif kc == 3:
                    q_block(hp + 1, wqj, Qpn, 1)
                elif kc == 4:
                    k_block(hp + 1, wkj, Kpn, 2)
                elif kc == 5:
                    k_block(hp + 1, wkj, Kpn, 3)
            # prefetch tail weights / residual stream during attention
            if kc == 14:
                if hp == 3:
                    nc.sync.dma_start(
                        out=xq_t, in_=io["xtq"].rearrange("c p t -> p c t"))
                elif hp == 5:
                    nc.sync.dma_start(
                        out=wpj,
                        in_=io["wproj"].rearrange("j p c q -> p j c q"))
            scores_exp(hp, kc, pts, Kp, Qp)
            if hp + 1 < DC and kc == 0:
                pts_next = [ppt.tile([P, TKC, Tq], F8, tag="pt",
                                     name=f"pt{hp + 1}_{lo}")
                            for lo in (0, 1)]
        if hp + 1 < DC:
            kq_next = (Kpn, Qpn)
        pts_prev, pts = pts, (pts_next if hp + 1 < DC else None)
        kq = kq_next

    # tail of the attention pipeline: last pair's PV + normalization
    povs[0] = pv_chain(DC - 1, 0, 0, pts_prev, ppo)
    povs[1] = pv_chain(DC - 1, 1, 0, pts_prev, ppo)
    pv_finish(DC - 1, 0, povs, pprb)
    povs[0] = pv_chain(DC - 1, 0, 1, pts_prev, ppo)
    povs[1] = pv_chain(DC - 1, 1, 1, pts_prev, ppo)
    pv_finish(DC - 1, 1, povs, pprb)

    pprb.release()
    ppo.release()
    pps.release()
    ppmm.release()
    ppt.release()
    pvaug.release()
    pwkv.release()
    pQp.release()
    pKp.release()
    pcol.release()
    prbm.release()
    pwv.release()
    pxT.release()

    # ============ tail: proj -> LN2 -> FFN, pipelined per n-block ========
    # cen reuses xq_t's storage (free after that block's proj drains) and
    # hsq reuses g_t's (free once the block's stats matmuls are done).
    pf1 = tc.alloc_tile_pool(name="pf1", bufs=1)
    pw1 = tc.alloc_tile_pool(name="pw1", bufs=1)
    pw2 = tc.alloc_tile_pool(name="pw2", bufs=8)
    f1g = pf1.tile([P, FC, Tq], F8, name="f1g")
    w1_all = pw1.tile([P, FC, 2 * NK1, P], F8, name="w1_all")
    w1r = io["w1"].rearrange("j p c q -> p j c q")
    w2_tiles = []
    for h2 in range(2):
        nc.sync.dma_start(out=w1_all[:, ts(h2, FC // 2)],
                          in_=w1r[:, ts(h2, FC // 2)])
        for jw in range(4 * h2, 4 * h2 + 4):
            w2t = pw2.tile([P, 2 * NK2, P], F8, tag="w2", name=f"w2t{jw}")
            nc.sync.dma_start(out=w2t, in_=io["w2"][jw])
            w2_tiles.append(w2t)
    w1_tiles = [w1_all[:, j] for j in range(FC)]

    ppE = tc.alloc_tile_pool(name="ppE", bufs=2, space="PSUM")
    ppst2 = tc.alloc_tile_pool(name="ppst2", bufs=2, space="PSUM")
    ppbc2 = tc.alloc_tile_pool(name="ppbc2", bufs=2, space="PSUM")
    hsq = g_t
    cen = xq_t

    for n in range(NQ):
        sl = ts(n, 512)
        for j in range(DC):
            psn = ppE.tile([P, 512], F32, tag="mm", name=f"psp{j}_{n}")
            for c in range(DC // 2):
                nc.tensor.matmul(psn, wpj[:, j, 2 * c:2 * c + 2, :],
                                 och_t[:, 2 * c:2 * c + 2, sl],
                                 start=(c == 0), stop=(c == DC // 2 - 1),
                                 perf_mode=DR)
            nc.vector.scalar_tensor_tensor(
                out=h_t[:, j, sl], in0=psn, scalar=inv8_s,
                in1=xq_t[:, j, sl], op0=ALU.mult, op1=ALU.add)
            if nzproj:
                nc.vector.tensor_scalar_add(h_t[:, j, sl], h_t[:, j, sl],
                                            bproj_s[:, j:j + 1])
    for n in range(NQ):
        sl = ts(n, 512)
        # ---- LN2 for this query block ----
        for c in range(DC):
            if c % 2 == 0:
                nc.scalar.square(hsq[:, c, sl], h_t[:, c, sl])
            else:
                nc.gpsimd.tensor_mul(hsq[:, c, sl], h_t[:, c, sl],
                                     h_t[:, c, sl])
        ps_mu = ppst2.tile([1, 512], F32, tag="st", name=f"ps2mu{n}")
        for c in range(DC):
            nc.tensor.matmul(ps_mu, invDb, h_t[:, c, sl],
                             start=(c == 0), stop=(c == DC - 1))
        ps_sq = ppst2.tile([1, 512], F32, tag="st", name=f"ps2sq{n}")
        for c in range(DC):
            nc.tensor.matmul(ps_sq, ones1_f8, hsq[:, c, sl],
                             start=(c == 0), stop=(c == DC - 1))
        mu2 = prows.tile([1, 512], F32, tag="mu2", name=f"mu2_{n}")
        nc.scalar.copy(out=mu2, in_=ps_mu)
        musq = prows.tile([1, 512], F32, tag="msq", name=f"musq2{n}")
        nc.vector.tensor_mul(musq, mu2, mu2)
        nc.vector.scalar_tensor_tensor(out=musq, in0=ps_sq, scalar=invD1,
                                       in1=musq, op0=ALU.mult,
                                       op1=ALU.subtract)
        sd = prows.tile([1, 512], F32, tag="mu", name=f"sd2{n}")
        nc.scalar.activation(out=sd, in_=musq, func=AF.Sqrt, bias=eps_t,
                             scale=1.0)
        r_f = prows.tile([1, 512], F32, tag="msq", name=f"r2{n}")
        nc.vector.reciprocal(out=r_f, in_=sd)
        rrow = prows.tile([1, 512], BF16, tag="rr", name=f"r2row{n}")
        nc.scalar.copy(out=rrow, in_=r_f)
        rmurow = prows.tile([1, 512], BF16, tag="rmr", name=f"rmu2row{n}")
        nc.gpsimd.tensor_mul(rmurow, mu2, rrow)
        bp1 = ppbc2.tile([P, 512], F32, tag="bc", name=f"b2p1{n}")
        nc.tensor.matmul(bp1, onesK1, rrow)
        nc.scalar.copy(out=r2B[:, sl], in_=bp1)
        bp2 = ppbc2.tile([P, 512], F32, tag="bc", name=f"b2p2{n}")
        nc.tensor.matmul(bp2, onesK1, rmurow)
        nc.scalar.copy(out=mur2B[:, sl], in_=bp2)
        for c in range(DC):
            eng = nc.gpsimd if c % 4 == 1 else nc.vector
            eng.tensor_mul(cen[:, c, sl], h_t[:, c, sl], r2B[:, sl])
            eng.tensor_sub(cen[:, c, sl], cen[:, c, sl], mur2B[:, sl])
            nc.scalar.activation(out=g_t[:, c, sl], in_=cen[:, c, sl],
                                 func=AF.Gelu, bias=bln2_s[:, c:c + 1],
                                 scale=g2_s[:, c:c + 1])

    ppbc2.release()
    ppst2.release()
    ppE.release()

    # ============ FFN1 + lagged early FFN2 chains for 2 of the j's =====
    NJE = 2
    NKK = FC // 2
    poutc = tc.alloc_tile_pool(name="poutc", bufs=3)
    ppG = tc.alloc_tile_pool(name="ppG", bufs=3, space="PSUM")
    ppGe = tc.alloc_tile_pool(name="ppGe", bufs=NJE, space="PSUM")

    psf = {}

    def f1_chain(j, n):
        if n == 0:
            psf[j] = ppG.tile([P, Tq], F32, tag="mm", name=f"psf{j}")
        for k in range(NK1):
            kr = k % (DC // 2)
            nc.tensor.matmul(psf[j][:, ts(n, 512)],
                             w1_tiles[j][:, 2 * k:2 * k + 2, :],
                             g_t[:, 2 * kr:2 * kr + 2, ts(n, 512)],
                             start=(k == 0), stop=(k == NK1 - 1),
                             perf_mode=DR)
        if n == 1:
            nc.scalar.activation(out=f1g[:, j, :], in_=psf.pop(j),
                                 func=AF.Gelu, bias=b1_s[:, j:j + 1],
                                 scale=1.0 / W1_SCALE)

    def pso_step(psj, j, k, n, start, stop):
        kr = k % (FC // 2)
        nc.tensor.matmul(psj, w2_tiles[j][:, 2 * k:2 * k + 2, :],
                         f1g[:, 2 * kr:2 * kr + 2, ts(n, 512)],
                         start=start, stop=stop, perf_mode=DR)

    def outc_store(psj, j, n):
        outc = poutc.tile([P, 512], F32, tag="oc", name=f"outc{j}_{n}")
        nc.vector.scalar_tensor_tensor(
            out=outc, in0=psj, scalar=inv_w2_s,
            in1=h_t[:, j, ts(n, 512)], op0=ALU.mult, op1=ALU.add)
        if nzb2:
            nc.vector.tensor_scalar_add(outc, outc, b2_s[:, j:j + 1])
        nc.sync.dma_start(out=io["out"][j][:, ts(n, 512)], in_=outc)

    psoE = [ppGe.tile([P, 512], F32, tag="me", name=f"psoE{je}")
            for je in range(NJE)]
    nxt = [0]

    def emit_steps(limit_kk):
        while nxt[0] < min(limit_kk, NKK):
            kk = nxt[0]
            last = kk == NKK - 1
            for je in range(NJE):
                pso_step(psoE[je], je, kk, 0, start=(kk == 0),
                         stop=(last and not W2_COMP))
            if W2_COMP:
                for je in range(NJE):
                    pso_step(psoE[je], je, NKK + kk, 0, start=False,
                             stop=last)
            nxt[0] += 1

    LAG = 6
    LEAD = 2
    for j in range(LEAD):
        f1_chain(j, 0)
    for j in range(LEAD, FC + LEAD):
        if j < FC:
            f1_chain(j, 0)
        f1_chain(j - LEAD, 1)
        emit_steps((j - LEAD - LAG) // 2)
    emit_steps(NKK)
    for je in range(NJE):
        outc_store(psoE[je], je, 0)
    ppGe.release()
    ppG.release()

    # ============ FFN2 remainder + residual + store ============
    ppG2 = tc.alloc_tile_pool(name="ppG2", bufs=4, space="PSUM")
    for n in range(NQ):
        for j in range(DC):
            if n == 0 and j < NJE:
                continue
            psj = ppG2.tile([P, 512], F32, tag="mo", name=f"pso{j}_{n}")
            for k in range(NK2):
                pso_step(psj, j, k, n, start=(k == 0), stop=(k == NK2 - 1))
            outc_store(psj, j, n)

    ppG2.release()
    poutc.release()
    pw2.release()
    pw1.release()
    pf1.release()
    pwproj.release()
    pxq.release()
    pr2.release()
    pg.release()
    ph.release()
    prc.release()
    prows.release()
    consts.release()


# ----------------------------------------------------------------------------
# host side
# ----------------------------------------------------------------------------

def _stripe(v):
    """[n*P] -> [P, n] per-partition striping (feature f = c*P + p)."""
    v = np.asarray(v, np.float32)
    return np.ascontiguousarray(v.reshape(-1, P).T)


def _lhsT_stream8(W, scale):
    """fp8e4m3 lhsT stream of W*scale (scale undone on-device)."""
    din, dout = W.shape
    r = (W * scale).astype(f8e4).reshape(din // P, P, dout // P, P)
    return np.ascontiguousarray(r.transpose(2, 1, 0, 3))


def _lhsT_stream8c(W, scale):
    """Compensated fp8 lhsT stream: [Dout/P, P, 2*Din/P, P] with hi rounds
    first, then the fp8-quantized residuals at the same scale (residuals land
    in e4m3's subnormal range whose fixed absolute step makes W-quant error
    negligible). Device accumulates both chains in one PSUM group."""
    din, dout = W.shape
    r = (np.asarray(W, np.float32) * scale)
    hi = r.astype(f8e4)
    lo = (r - hi.astype(np.float32)).astype(f8e4)
    full = np.concatenate([hi.reshape(din // P, P, dout // P, P),
                           lo.reshape(din // P, P, dout // P, P)], axis=0)
    return np.ascontiguousarray(full.transpose(2, 1, 0, 3))


def prep_shared(inputs):
    f32 = np.float32
    g1 = np.asarray(inputs["ln1_g"], f32)
    b1n = np.asarray(inputs["ln1_b"], f32)
    W_ap = np.asarray(inputs["W_ap"], f32)
    b_ap = np.asarray(inputs["b_ap"], f32)
    W_qkv = np.asarray(inputs["W_qkv"], f32)
    b_qkv = np.asarray(inputs["b_qkv"], f32)
    W_proj = np.asarray(inputs["W_proj"], f32)

    # fold LN1 gamma and the whole attn pre-projection into W_qkv:
    # qkv = ln1(x) @ W_ap' @ W_qkv + (b_ap' @ W_qkv + b_qkv)
    W_eff = (g1[:, None] * W_ap) @ W_qkv
    b_eff = (b_ap + b1n @ W_ap) @ W_qkv + b_qkv
    shared = {
        "wq": _lhsT_stream8(W_eff[:, 0:D], WQ_SCALE),
        "wk": _lhsT_stream8(W_eff[:, D:2 * D], WQ_SCALE),
        "wv": np.ascontiguousarray(
            (W_eff[:, 2 * D:] * WQ_SCALE).astype(f8e4).reshape(DC, P, D)),
        "bqkv": _stripe(b_eff[:2 * D]),
        "bv": np.ascontiguousarray(np.asarray(b_eff[2 * D:], f32)),
        "wproj": _lhsT_stream8(W_proj, WQ_SCALE),
        "bproj": _stripe(np.asarray(inputs["b_proj"], f32)),
        "w1": (_lhsT_stream8c if W1_COMP else _lhsT_stream8)(
            np.asarray(inputs["W1"], f32), W1_SCALE),
        "b1": _stripe(np.asarray(inputs["b1"], f32)),
        "w2": (_lhsT_stream8c if W2_COMP else _lhsT_stream8)(
            np.asarray(inputs["W2"], f32), W2_SCALE),
        "b2": _stripe(np.asarray(inputs["b2"], f32)),
        "g2": _stripe(np.asarray(inputs["ln2_g"], f32)),
        "bln2": _stripe(np.asarray(inputs["ln2_b"], f32)),
    }
    return shared


def nz_flags(inputs):
    f32 = np.float32
    g1 = np.asarray(inputs["ln1_g"], f32)
    b1n = np.asarray(inputs["ln1_b"], f32)
    W_ap = np.asarray(inputs["W_ap"], f32)
    b_ap = np.asarray(inputs["b_ap"], f32)
    W_qkv = np.asarray(inputs["W_qkv"], f32)
    b_eff = (b_ap + b1n @ W_ap) @ W_qkv + np.asarray(inputs["b_qkv"], f32)
    return dict(
        nzqkv=bool(np.any(b_eff[:2 * D])),
        nzproj=bool(np.any(np.asarray(inputs["b_proj"]))),
        nzb2=bool(np.any(np.asarray(inputs["b2"]))),
    )


def prep_core_x(x, core):
    b, qh = core // 2, core % 2
    xTb = np.asarray(x[b], np.float32).T  # [D, T] view
    if qh:
        xTb = np.concatenate([xTb[:, Tq:], xTb[:, :Tq]], axis=1)
    return {
        "xt": np.ascontiguousarray(xTb.astype(f8e4).reshape(DC, P, T)),
        "xtq": np.ascontiguousarray(
            xTb[:, :Tq].astype(bf16).reshape(DC, P, Tq)),
    }


def assemble_output(results, dtype):
    out = np.empty((B, T, D), dtype)
    for c in range(N_CORES):
        b, qh = c // 2, c % 2
        arr = np.asarray(results[c]["out"]).reshape(D, Tq)
        out[b, qh * Tq:(qh + 1) * Tq, :] = arr.T
    return out


def kernel(**inputs):
    x = np.asarray(inputs["x"], np.float32)
    shared = prep_shared(inputs)
    nc = build_nc(**nz_flags(inputs))
    in_maps = [dict(shared, **prep_core_x(x, c)) for c in range(N_CORES)]
    res = run_bass_kernel_spmd(nc, in_maps, list(range(N_CORES)))
    return assemble_output(res.results, np.float32)


if __name__ == "__main__":
    nc = build_nc()
    print("built ok")


# revision 44
# speedup vs baseline: 1.0061x; 1.0023x over previous
"""Trainium2 Bass kernel for a dense transformer block (nn_Block_7713761264306).

Sharding: 8 cores = 4 batches x 2 query-halves. Each core computes K/V over the
full sequence for its batch, but runs only its 1024 query rows through
attention and the FFN. The query half is selected by rotating the token axis
host-side (exact: no mask, softmax is permutation-invariant over keys).
No collectives.

Device layout: activations are kept feature-on-partition ([D, tokens]) so every
linear layer is a direct PE matmul (lhsT = weights, rhs = activations^T) with
no on-device transposes. LayerNorm is folded into the matmul pipeline: x is
centered in place (xc = x - mu, mu via fp8 ones-matmul stats), and the
per-token 1/sd is applied at each QKV PSUM drain (row-broadcast tile for K/Q,
a transposed per-token column - obtained via a tiny DRAM round trip - for V).
This removes the normalized-x materialization from the critical path entirely;
QKV matmuls run directly on xc while later column blocks are still being
centered. Softmax runs in S^T layout (keys on partitions, queries free);
denominators come from a ones-column appended to V in the PV matmul. LN1's
gamma and the attn pre-projection fold into W_qkv host-side.

The whole kernel is software-pipelined at emission level (engines execute
their streams in order): LN1 stats/centering per 512-column block interleave
with the first head-pair's K/Q/scores; each attention pair's exp stream (the
ACT bottleneck, ~66% of all cycles) overlaps the previous pair's PV/output
normalization and the next pair's K/Q production; V production hides in the
first pair's exp window; the FFN tail pipelines proj -> LN2 -> FFN1 -> FFN2
per 512-query block with n0-leading chain emission.

fp8: QKV/proj/PV/FFN matmuls run as fp8e4m3 DoubleRow (2 K-chunks per
instruction, 0.5 cyc/row). Weights are pre-scaled (x8/x16/x32) out of e4m3's
subnormal range and unscaled via free activation/stt scalar slots; W2
additionally carries an fp8 residual chain (hi+lo) so its quantization error
is negligible (W1's is dropped - its error hides under the fp8 activation
noise floor). exp outputs fp8 directly with a -2 bias shift (uniform factor
cancels in the softmax normalization) centering probs in e4m3 range. Scores
stay bf16. The residual stream (xtq) stays bf16.
"""

import numpy as np
import ml_dtypes

import concourse.bass as bass
import concourse.mybir as mybir
import concourse.tile as tile
from concourse.bass import ts
from concourse.bass_utils import run_bass_kernel_spmd

BF16 = mybir.dt.bfloat16
F32 = mybir.dt.float32
F8 = mybir.dt.float8e4
bf16 = ml_dtypes.bfloat16
f8e4 = ml_dtypes.float8_e4m3
DR = mybir.MatmulPerfMode.DoubleRow

# fp8 weight pre-scales (undone on-device via free scale slots). The raw
# weight sigmas (~1/32, ~1/64) sit in e4m3's subnormal range; scaling up
# recovers full mantissa precision.
W1_SCALE = 16.0
W2_SCALE = 32.0
WQ_SCALE = 8.0       # W_eff / W_proj pre-scale
ESHIFT = -2.0        # exp bias shift: centers softmax numerators in e4m3
W1_COMP = False      # fp8 residual (hi+lo) chain for W1
W2_COMP = True       # fp8 residual (hi+lo) chain for W2

B, T, D, H, HS, FF = 4, 2048, 1024, 16, 64, 4096
P = 128
DC = D // P          # 8 feature chunks
FC = FF // P         # 32 ffn chunks
TKC = T // P         # 16 key/token chunks
Tq = 1024            # queries per core
NT = T // 512        # 4 column blocks over full seq
NQ = Tq // 512       # 2 column blocks over queries
N_CORES = 8
EPS = 1e-5
NK1 = DC if W1_COMP else DC // 2
NK2 = FC if W2_COMP else FC // 2

AF = mybir.ActivationFunctionType
ALU = mybir.AluOpType


def build_nc(cap=True, nzqkv=False, nzproj=False, nzb2=False):
    nc = bass.Bass()
    io = {}
    io["xt"] = nc.dram_tensor("xt", [DC, P, T], F8, kind="ExternalInput")
    io["xtq"] = nc.dram_tensor("xtq", [DC, P, Tq], BF16,
                               kind="ExternalInput")
    io["wq"] = nc.dram_tensor("wq", [DC, P, DC, P], F8, kind="ExternalInput")
    io["wk"] = nc.dram_tensor("wk", [DC, P, DC, P], F8, kind="ExternalInput")
    io["wv"] = nc.dram_tensor("wv", [DC, P, D], F8, kind="ExternalInput")
    io["bqkv"] = nc.dram_tensor("bqkv", [P, 2 * DC], F32, kind="ExternalInput")
    io["bv"] = nc.dram_tensor("bv", [D], BF16, kind="ExternalInput")
    io["wproj"] = nc.dram_tensor("wproj", [DC, P, DC, P], F8,
                                 kind="ExternalInput")
    io["bproj"] = nc.dram_tensor("bproj", [P, DC], F32, kind="ExternalInput")
    io["w1"] = nc.dram_tensor("w1", [FC, P, 2 * NK1, P], F8,
                              kind="ExternalInput")
    io["b1"] = nc.dram_tensor("b1", [P, FC], F32, kind="ExternalInput")
    io["w2"] = nc.dram_tensor("w2", [DC, P, 2 * NK2, P], F8,
                              kind="ExternalInput")
    io["b2"] = nc.dram_tensor("b2", [P, DC], F32, kind="ExternalInput")
    io["g2"] = nc.dram_tensor("g2", [P, DC], F32, kind="ExternalInput")
    io["bln2"] = nc.dram_tensor("bln2", [P, DC], F32, kind="ExternalInput")
    io["out"] = nc.dram_tensor("out", [DC, P, Tq], F32, kind="ExternalOutput")
    io["scr"] = nc.dram_tensor("scr", [NT, 512], BF16, kind="Internal")

    with tile.TileContext(nc) as tc:
        _emit(nc, tc, io, nzqkv, nzproj, nzb2)
    nc.finalize()
    if cap:
        _cap_waits(nc)
    return nc


def _cap_waits(nc, keep_types=()):
    """This toolchain's walrus accepts only one sync-wait command per compute
    instruction; hoist extra waits into preceding same-engine NoOps."""
    cnt = 0
    for fn in nc.m.functions:
        for blk in fn.blocks:
            new = []
            for inst in blk.instructions:
                si = getattr(inst, "sync_info", None)
                if si is not None and len(si.on_wait) > 1 \
                        and type(inst).__name__ not in keep_types:
                    waits = list(si.on_wait)
                    for w in waits[:-1]:
                        cnt += 1
                        nop = mybir.InstNoOp(
                            name=f"{inst.name}-w{cnt}", ins=[], outs=[])
                        nop.engine = inst.engine
                        nop.sync_info = mybir.SyncInfo(on_wait=[w],
                                                       on_update=[])
                        new.append(nop)
                    inst.sync_info = mybir.SyncInfo(
                        on_wait=[waits[-1]], on_update=list(si.on_update))
                new.append(inst)
            blk.instructions = new
    return cnt


def _emit(nc, tc, io, nzqkv=False, nzproj=False, nzb2=False):
    # Pools release in LIFO order: tail-lived pools sit at the stack bottom,
    # attention-lived above them, phase-A-only PSUM pools on top.
    consts = tc.alloc_tile_pool(name="consts", bufs=1)
    prows = tc.alloc_tile_pool(name="prows", bufs=1)
    prc = tc.alloc_tile_pool(name="prc", bufs=2)
    ph = tc.alloc_tile_pool(name="ph", bufs=1)
    pg = tc.alloc_tile_pool(name="pg", bufs=1)
    pr2 = tc.alloc_tile_pool(name="pr2", bufs=1)
    pxq = tc.alloc_tile_pool(name="pxq", bufs=1)
    pwproj = tc.alloc_tile_pool(name="pwproj", bufs=1)

    # ---------------- attention-lived SBUF pools ----------------
    pxT = tc.alloc_tile_pool(name="pxT", bufs=1)
    pwv = tc.alloc_tile_pool(name="pwv", bufs=1)
    prbm = tc.alloc_tile_pool(name="prbm", bufs=1)
    pcol = tc.alloc_tile_pool(name="pcol", bufs=1)
    pKp = tc.alloc_tile_pool(name="pKp", bufs=2)
    pQp = tc.alloc_tile_pool(name="pQp", bufs=2)
    pwkv = tc.alloc_tile_pool(name="pwkv", bufs=3)
    pvaug = tc.alloc_tile_pool(name="pvaug", bufs=1)
    ppt = tc.alloc_tile_pool(name="ppt", bufs=4)

    # PSUM pools for phase A + attention (8 banks exactly; st/bc release
    # after LN1 block 3 frees banks for V, whose release frees po/rbp).
    ppmm = tc.alloc_tile_pool(name="ppmm", bufs=1, space="PSUM")
    pps = tc.alloc_tile_pool(name="pps", bufs=2, space="PSUM")
    pxsq = tc.alloc_tile_pool(name="pxsq", bufs=1)
    ppst = tc.alloc_tile_pool(name="ppst", bufs=2, space="PSUM")
    ppbc = tc.alloc_tile_pool(name="ppbc", bufs=1, space="PSUM")

    # ------------------------- DMAs first (critical path) ----------------
    xT = pxT.tile([P, DC, T], F8, name="xT")
    xt_r = io["xt"].rearrange("c p t -> p c t")
    for kt in range(NT):
        nc.sync.dma_start(out=xT[:, :, ts(kt, 512)],
                          in_=xt_r[:, :, ts(kt, 512)])

    wkj0 = pwkv.tile([P, DC, P], F8, tag="w", name="wkj0")
    nc.sync.dma_start(out=wkj0, in_=io["wk"][0])
    wqj0 = pwkv.tile([P, DC, P], F8, tag="w", name="wqj0")
    nc.sync.dma_start(out=wqj0, in_=io["wq"][0])
    wv_t = pwv.tile([P, DC, D], F8, name="wv_t")
    nc.sync.dma_start(out=wv_t, in_=io["wv"].rearrange("c p d -> p c d"))

    bvB = consts.tile([P, D], BF16)
    nc.sync.dma_start(out=bvB, in_=io["bv"][:].partition_broadcast(P))
    b1_s = consts.tile([P, FC], F32)
    nc.sync.dma_start(out=b1_s, in_=io["b1"][:])
    g2_s = consts.tile([P, DC], F32)
    nc.sync.dma_start(out=g2_s, in_=io["g2"][:])
    bln2_s = consts.tile([P, DC], F32)
    nc.sync.dma_start(out=bln2_s, in_=io["bln2"][:])
    if nzqkv:
        bqkv_s = consts.tile([P, 2 * DC], F32)
        nc.sync.dma_start(out=bqkv_s, in_=io["bqkv"][:])
    if nzproj:
        bproj_s = consts.tile([P, DC], F32)
        nc.sync.dma_start(out=bproj_s, in_=io["bproj"][:])
    if nzb2:
        b2_s = consts.tile([P, DC], F32)
        nc.sync.dma_start(out=b2_s, in_=io["b2"][:])

    # ------------------------- constants -------------------------
    inv_w2_s = consts.tile([P, 1], F32)
    nc.vector.memset(inv_w2_s, 1.0 / W2_SCALE)
    inv8_s = consts.tile([P, 1], F32)
    nc.vector.memset(inv8_s, 1.0 / WQ_SCALE)
    esh_s = consts.tile([P, 1], F32)
    nc.vector.memset(esh_s, ESHIFT)
    invD1 = consts.tile([1, 1], F32)
    nc.vector.memset(invD1, 1.0 / D)
    invDb = consts.tile([P, 1], BF16)
    nc.vector.memset(invDb, 1.0 / D)
    ones1_f8 = consts.tile([P, 1], F8)
    nc.vector.memset(ones1_f8, 1.0)
    onesK1 = consts.tile([1, P], BF16)
    nc.vector.memset(onesK1, 1.0)
    eps_t = consts.tile([1, 1], F32)
    nc.vector.memset(eps_t, EPS)

    rB = prbm.tile([P, T], BF16, tag="rb", name="rB")
    muB = prbm.tile([P, T], BF16, tag="mb", name="muB")
    rcol = pcol.tile([P, TKC], BF16, name="rcol")
    v_aug = pvaug.tile([P, TKC, H * (HS + 1)], F8, name="v_aug")
    v4 = v_aug.rearrange("p i (h e) -> p i h e", e=HS + 1)
    nc.vector.memset(v4[:, :, :, HS:HS + 1], 1.0)
    bv4 = bvB.rearrange("p (c d) -> p c d", d=HS)
    xq_t = pxq.tile([P, DC, Tq], BF16, name="xq_t")
    wpj = pwproj.tile([P, DC, DC, P], F8, name="wpj")
    h_t = ph.tile([P, DC, Tq], BF16, name="h_t")
    g_t = pg.tile([P, DC, Tq], F8, name="g_t")
    och_t = g_t
    r2B = pr2.tile([P, Tq], BF16, tag="rb", name="r2B")
    mur2B = pr2.tile([P, Tq], BF16, tag="mb", name="mur2B")

    def ln1_block(kt, on_act):
        """Squares, stats, row math, broadcasts, and in-place centering for
        one 512-column block; also writes this block's r/8 row to scr."""
        sl = ts(kt, 512)
        xsq = pxsq.tile([P, DC, 512], F8, tag="xs", name=f"xsq{kt}")
        for c in range(DC):
            if on_act and c < 4:
                nc.scalar.square(xsq[:, c, :], xT[:, c, sl])
            elif c % 2 == 0:
                nc.vector.tensor_mul(xsq[:, c, :], xT[:, c, sl],
                                     xT[:, c, sl])
            else:
                nc.gpsimd.tensor_mul(xsq[:, c, :], xT[:, c, sl],
                                     xT[:, c, sl])
        ps_mu = ppst.tile([1, 512], F32, tag="st", name=f"psmu{kt}")
        for c in range(DC):
            nc.tensor.matmul(ps_mu, ones1_f8, xT[:, c, sl],
                             start=(c == 0), stop=(c == DC - 1))
        ps_sq = ppst.tile([1, 512], F32, tag="st", name=f"pssq{kt}")
        for c in range(DC):
            nc.tensor.matmul(ps_sq, ones1_f8, xsq[:, c, :],
                             start=(c == 0), stop=(c == DC - 1))
        mu_f = prows.tile([1, 512], F32, tag="mu", name=f"mu{kt}")
        nc.vector.tensor_scalar_mul(mu_f, ps_mu, 1.0 / D)
        mubf = prows.tile([1, 512], BF16, tag="mub", name=f"mubf{kt}")
        nc.scalar.copy(out=mubf, in_=mu_f)
        musq = prows.tile([1, 512], F32, tag="msq", name=f"musq{kt}")
        nc.vector.tensor_mul(musq, mu_f, mu_f)
        nc.vector.scalar_tensor_tensor(out=musq, in0=ps_sq, scalar=invD1,
                                       in1=musq, op0=ALU.mult,
                                       op1=ALU.subtract)
        sd = prows.tile([1, 512], F32, tag="mu", name=f"sd{kt}")
        nc.scalar.activation(out=sd, in_=musq, func=AF.Sqrt, bias=eps_t,
                             scale=1.0)
        r_f = prows.tile([1, 512], F32, tag="msq", name=f"r{kt}")
        nc.vector.reciprocal(out=r_f, in_=sd)
        rrow = prows.tile([1, 512], BF16, tag="rr", name=f"rrow{kt}")
        nc.vector.tensor_scalar_mul(rrow, r_f, 1.0 / WQ_SCALE)
        nc.sync.dma_start(out=io["scr"][kt:kt + 1, :], in_=rrow)
        bp1 = ppbc.tile([P, 512], F32, tag="bc", name=f"bp1{kt}")
        nc.tensor.matmul(bp1, onesK1, rrow)
        cpy = nc.scalar.copy if on_act else nc.vector.tensor_copy
        cpy(out=rB[:, sl], in_=bp1)
        bp2 = ppbc.tile([P, 512], F32, tag="bc", name=f"bp2{kt}")
        nc.tensor.matmul(bp2, onesK1, mubf)
        cpy(out=muB[:, sl], in_=bp2)
        # center x in place: xc = x - mu
        for c in range(DC):
            eng = nc.vector if c % 2 == 0 else nc.gpsimd
            eng.tensor_sub(xT[:, c, sl], xT[:, c, sl], muB[:, sl])

    def k_block(hp, wkj, Kp, kt):
        sl = ts(kt, 512)
        psk = ppmm.tile([P, 512], F32, tag="mm", name=f"psk{hp}_{kt}")
        for c in range(DC // 2):
            nc.tensor.matmul(psk, wkj[:, 2 * c:2 * c + 2, :],
                             xT[:, 2 * c:2 * c + 2, sl],
                             start=(c == 0), stop=(c == DC // 2 - 1),
                             perf_mode=DR)
        nc.vector.tensor_mul(Kp[:, sl], psk, rB[:, sl])
        if nzqkv:
            nc.vector.tensor_scalar_add(Kp[:, sl], Kp[:, sl],
                                        bqkv_s[:, DC + hp:DC + hp + 1])

    def q_block(hp, wqj, Qp, kt):
        sl = ts(kt, 512)
        psq = ppmm.tile([P, 512], F32, tag="mm", name=f"psq{hp}_{kt}")
        for c in range(DC // 2):
            nc.tensor.matmul(psq, wqj[:, 2 * c:2 * c + 2, :],
                             xT[:, 2 * c:2 * c + 2, sl],
                             start=(c == 0), stop=(c == DC // 2 - 1),
                             perf_mode=DR)
        nc.vector.tensor_mul(Qp[:, sl], psq, rB[:, sl])
        if nzqkv:
            nc.vector.tensor_scalar_add(Qp[:, sl], Qp[:, sl],
                                        bqkv_s[:, hp:hp + 1])

    def v_chunk(i, ppv):
        ps = [ppv.tile([P, 512], F32, tag="vps", name=f"psv{i}_{n}")
              for n in range(NQ)]
        for c in range(DC // 2):
            for n in range(NQ):
                nc.tensor.matmul(ps[n], xT[:, 2 * c:2 * c + 2, ts(i, P)],
                                 wv_t[:, 2 * c:2 * c + 2, ts(n, 512)],
                                 start=(c == 0), stop=(c == DC // 2 - 1),
                                 perf_mode=DR)
        for n in range(NQ):
            nc.vector.scalar_tensor_tensor(
                out=v4[:, i, n * DC:(n + 1) * DC, 0:HS],
                in0=ps[n].rearrange("p (h d) -> p h d", d=HS),
                scalar=rcol[:, i:i + 1],
                in1=bv4[:, n * DC:(n + 1) * DC, :],
                op0=ALU.mult, op1=ALU.add)

    def scores_exp(hp, kc, pts, Kp, Qp):
        for lo in (0, 1):
            ps = pps.tile([P, Tq], F32, tag="s", name=f"pss{hp}_{kc}_{lo}")
            for n in range(NQ):
                nc.tensor.matmul(ps[:, ts(n, 512)],
                                 Kp[lo * HS:(lo + 1) * HS, ts(kc, P)],
                                 Qp[lo * HS:(lo + 1) * HS, ts(n, 512)])
            nc.scalar.activation(out=pts[lo][:, kc, :], in_=ps, func=AF.Exp,
                                 bias=esh_s, scale=float(1.0 / np.sqrt(HS)))

    def pv_chain(hp, lo, qt, pts, ppo):
        h = 2 * hp + lo
        po = ppo.tile([HS + 1, 512], F32, tag="po", name=f"po{h}_{qt}")
        for kc in range(TKC // 2):
            nc.tensor.matmul(po,
                             v_aug[:, 2 * kc:2 * kc + 2,
                                   h * (HS + 1):(h + 1) * (HS + 1)],
                             pts[lo][:, 2 * kc:2 * kc + 2, ts(qt, 512)],
                             start=(kc == 0), stop=(kc == TKC // 2 - 1),
                             perf_mode=DR)
        rc = prc.tile([1, 512], F32, tag="rc", name=f"rc{h}_{qt}")
        nc.vector.reciprocal(out=rc, in_=po[HS:HS + 1, :])
        rcb = prc.tile([1, 512], BF16, tag="rcb", name=f"rcb{h}_{qt}")
        nc.vector.tensor_copy(out=rcb, in_=rc)
        return po, rcb

    def pv_finish(hp, qt, povs, pprb):
        rbp = pprb.tile([P, 512], F32, tag="rbp", name=f"rbp{hp}_{qt}")
        for lo in (0, 1):
            nc.tensor.matmul(rbp[lo * HS:(lo + 1) * HS, :], onesK1[:, 0:HS],
                             povs[lo][1])
        rbs = prc.tile([P, 512], BF16, tag="rbs", name=f"rbs{hp}_{qt}")
        nc.vector.tensor_copy(out=rbs, in_=rbp)
        for lo in (0, 1):
            nc.vector.tensor_mul(och_t[lo * HS:(lo + 1) * HS, hp,
                                       ts(qt, 512)],
                                 povs[lo][0][0:HS, :],
                                 rbs[lo * HS:(lo + 1) * HS, :])

    # ================= phase A: blocks 0,1 then pipelined ==============
    ln1_block(0, on_act=True)
    ln1_block(1, on_act=True)
    Kp0 = pKp.tile([P, T], BF16, tag="kp", name="kp0")
    Qp0 = pQp.tile([P, Tq], BF16, tag="qp", name="qp0")
    k_block(0, wkj0, Kp0, 0)
    q_block(0, wqj0, Qp0, 0)
    k_block(0, wkj0, Kp0, 1)
    q_block(0, wqj0, Qp0, 1)

    pts0 = [ppt.tile([P, TKC, Tq], F8, tag="pt", name=f"pt0_{lo}")
            for lo in (0, 1)]
    scores_exp(0, 0, pts0, Kp0, Qp0)
    scores_exp(0, 1, pts0, Kp0, Qp0)
    ln1_block(2, on_act=False)
    k_block(0, wkj0, Kp0, 2)
    scores_exp(0, 2, pts0, Kp0, Qp0)
    scores_exp(0, 3, pts0, Kp0, Qp0)
    ln1_block(3, on_act=False)
    k_block(0, wkj0, Kp0, 3)
    ppbc.release()
    ppst.release()
    pxsq.release()
    # transposed r/8 per-token columns for the V drains
    nc.sync.dma_start(out=rcol.rearrange("p (kt j) -> p kt j", j=NT),
                      in_=io["scr"].rearrange("kt (j p) -> p kt j", p=P))
    ppv = tc.alloc_tile_pool(name="ppv", bufs=2, space="PSUM")

    # attention state for the software pipeline
    kq = (Kp0, Qp0)
    kq_next = None
    pts = pts0
    pts_prev = None
    ppo = None
    pprb = None
    povs = {}

    for hp in range(DC):
        Kp, Qp = kq
        if hp + 1 < DC:
            wkj = pwkv.tile([P, DC, P], F8, tag="w", name=f"wkj{hp + 1}")
            wqj = pwkv.tile([P, DC, P], F8, tag="w", name=f"wqj{hp + 1}")
            Kpn = pKp.tile([P, T], BF16, tag="kp", name=f"kp{hp + 1}")
            Qpn = pQp.tile([P, Tq], BF16, tag="qp", name=f"qp{hp + 1}")
        pv0 = 8 if hp == 1 else 0
        for kc in range(TKC):
            if hp > 1:
                if kc == 0:
                    povs[0] = pv_chain(hp - 1, 0, 0, pts_prev, ppo)
                elif kc == 1:
                    povs[1] = pv_chain(hp - 1, 1, 0, pts_prev, ppo)
                elif kc == 2:
                    pv_finish(hp - 1, 0, povs, pprb)
                elif kc == 3:
                    povs[0] = pv_chain(hp - 1, 0, 1, pts_prev, ppo)
                elif kc == 4:
                    povs[1] = pv_chain(hp - 1, 1, 1, pts_prev, ppo)
                elif kc == 5:
                    pv_finish(hp - 1, 1, povs, pprb)
            elif hp == 1:
                if kc < 8:
                    v_chunk(8 + kc, ppv)
                elif kc == 8:
                    ppv.release()
                    ppo = tc.alloc_tile_pool(name="ppo", bufs=2,
                                             space="PSUM")
                    pprb = tc.alloc_tile_pool(name="pprb", bufs=1,
                                              space="PSUM")
                    povs[0] = pv_chain(0, 0, 0, pts_prev, ppo)
                elif kc == 9:
                    povs[1] = pv_chain(0, 1, 0, pts_prev, ppo)
                elif kc == 10:
                    pv_finish(0, 0, povs, pprb)
                elif kc == 11:
                    povs[0] = pv_chain(0, 0, 1, pts_prev, ppo)
                elif kc == 12:
                    povs[1] = pv_chain(0, 1, 1, pts_prev, ppo)
                elif kc == 13:
                    pv_finish(0, 1, povs, pprb)
            elif hp == 0 and kc >= 8:
                v_chunk(kc - 8, ppv)
            if hp + 1 < DC:
                if kc == 0:
                    nc.sync.dma_start(out=wkj, in_=io["wk"][hp + 1])
                    nc.sync.dma_start(out=wqj, in_=io["wq"][hp + 1])
                elif hp == 0:
                    if kc == 10:
                        q_block(1, wqj, Qpn, 0)
                    elif kc == 11:
                        q_block(1, wqj, Qpn, 1)
                    elif kc >= 12:
                        k_block(1, wkj, Kpn, kc - 12)
                elif kc == 6 + pv0:
                    k_block(hp + 1, wkj, Kpn, 0)
                elif kc == 7 + pv0:
                    k_block(hp + 1, wkj, Kpn, 1)
                elif kc == 2:
                    q_block(hp + 1, wqj, Qpn, 0)
                elif kc == 3:
                    q_block(hp + 1, wqj, Qpn, 1)
                elif kc == 4:
                    k_block(hp + 1, wkj, Kpn, 2)
                elif kc == 5:
                    k_block(hp + 1, wkj, Kpn, 3)
            # prefetch tail weights / residual stream during attention
            if kc == 14:
                if hp == 3:
                    nc.sync.dma_start(
                        out=xq_t, in_=io["xtq"].rearrange("c p t -> p c t"))
                elif hp == 5:
                    nc.sync.dma_start(
                        out=wpj,
                        in_=io["wproj"].rearrange("j p c q -> p j c q"))
            scores_exp(hp, kc, pts, Kp, Qp)
            if hp + 1 < DC and kc == 0:
                pts_next = [ppt.tile([P, TKC, Tq], F8, tag="pt",
                                     name=f"pt{hp + 1}_{lo}")
                            for lo in (0, 1)]
        if hp + 1 < DC:
            kq_next = (Kpn, Qpn)
        pts_prev, pts = pts, (pts_next if hp + 1 < DC else None)
        kq = kq_next

    # tail of the attention pipeline: last pair's PV + normalization
    povs[0] = pv_chain(DC - 1, 0, 0, pts_prev, ppo)
    povs[1] = pv_chain(DC - 1, 1, 0, pts_prev, ppo)
    pv_finish(DC - 1, 0, povs, pprb)
    povs[0] = pv_chain(DC - 1, 0, 1, pts_prev, ppo)
    povs[1] = pv_chain(DC - 1, 1, 1, pts_prev, ppo)
    pv_finish(DC - 1, 1, povs, pprb)

    pprb.release()
    ppo.release()
    pps.release()
    ppmm.release()
    ppt.release()
    pvaug.release()
    pwkv.release()
    pQp.release()
    pKp.release()
    pcol.release()
    prbm.release()
    pwv.release()
    pxT.release()

    # ============ tail: proj -> LN2 -> FFN, pipelined per n-block ========
    # cen reuses xq_t's storage (free after that block's proj drains) and
    # hsq reuses g_t's (free once the block's stats matmuls are done).
    pf1 = tc.alloc_tile_pool(name="pf1", bufs=1)
    pw1 = tc.alloc_tile_pool(name="pw1", bufs=1)
    pw2 = tc.alloc_tile_pool(name="pw2", bufs=8)
    f1g = pf1.tile([P, FC, Tq], F8, name="f1g")
    w1_all = pw1.tile([P, FC, 2 * NK1, P], F8, name="w1_all")
    w1r = io["w1"].rearrange("j p c q -> p j c q")
    w2_tiles = []
    for h2 in range(2):
        nc.sync.dma_start(out=w1_all[:, ts(h2, FC // 2)],
                          in_=w1r[:, ts(h2, FC // 2)])
        for jw in range(4 * h2, 4 * h2 + 4):
            w2t = pw2.tile([P, 2 * NK2, P], F8, tag="w2", name=f"w2t{jw}")
            nc.sync.dma_start(out=w2t, in_=io["w2"][jw])
            w2_tiles.append(w2t)
    w1_tiles = [w1_all[:, j] for j in range(FC)]

    ppE = tc.alloc_tile_pool(name="ppE", bufs=2, space="PSUM")
    ppst2 = tc.alloc_tile_pool(name="ppst2", bufs=2, space="PSUM")
    ppbc2 = tc.alloc_tile_pool(name="ppbc2", bufs=2, space="PSUM")
    hsq = g_t
    cen = xq_t

    for n in range(NQ):
        sl = ts(n, 512)
        for j in range(DC):
            psn = ppE.tile([P, 512], F32, tag="mm", name=f"psp{j}_{n}")
            for c in range(DC // 2):
                nc.tensor.matmul(psn, wpj[:, j, 2 * c:2 * c + 2, :],
                                 och_t[:, 2 * c:2 * c + 2, sl],
                                 start=(c == 0), stop=(c == DC // 2 - 1),
                                 perf_mode=DR)
            nc.vector.scalar_tensor_tensor(
                out=h_t[:, j, sl], in0=psn, scalar=inv8_s,
                in1=xq_t[:, j, sl], op0=ALU.mult, op1=ALU.add)
            if nzproj:
                nc.vector.tensor_scalar_add(h_t[:, j, sl], h_t[:, j, sl],
                                            bproj_s[:, j:j + 1])
    for n in range(NQ):
        sl = ts(n, 512)
        # ---- LN2 for this query block: n0's squares lean on ACT (idle
        # then); n1's avoid ACT so its stats overlap gelu(n0) ----
        for c in range(DC):
            if n == 0 and c % 2 == 0:
                nc.scalar.square(hsq[:, c, sl], h_t[:, c, sl])
            elif (n == 1 and c % 2 == 0) or (n == 0 and c % 4 == 1):
                nc.vector.tensor_mul(hsq[:, c, sl], h_t[:, c, sl],
                                     h_t[:, c, sl])
            else:
                nc.gpsimd.tensor_mul(hsq[:, c, sl], h_t[:, c, sl],
                                     h_t[:, c, sl])
        ps_mu = ppst2.tile([1, 512], F32, tag="st", name=f"ps2mu{n}")
        for c in range(DC):
            nc.tensor.matmul(ps_mu, invDb, h_t[:, c, sl],
                             start=(c == 0), stop=(c == DC - 1))
        ps_sq = ppst2.tile([1, 512], F32, tag="st", name=f"ps2sq{n}")
        for c in range(DC):
            nc.tensor.matmul(ps_sq, ones1_f8, hsq[:, c, sl],
                             start=(c == 0), stop=(c == DC - 1))
        mu2 = prows.tile([1, 512], F32, tag="mu2", name=f"mu2_{n}")
        nc.scalar.copy(out=mu2, in_=ps_mu)
        musq = prows.tile([1, 512], F32, tag="msq", name=f"musq2{n}")
        nc.vector.tensor_mul(musq, mu2, mu2)
        nc.vector.scalar_tensor_tensor(out=musq, in0=ps_sq, scalar=invD1,
                                       in1=musq, op0=ALU.mult,
                                       op1=ALU.subtract)
        sd = prows.tile([1, 512], F32, tag="mu", name=f"sd2{n}")
        nc.scalar.activation(out=sd, in_=musq, func=AF.Sqrt, bias=eps_t,
                             scale=1.0)
        r_f = prows.tile([1, 512], F32, tag="msq", name=f"r2{n}")
        nc.vector.reciprocal(out=r_f, in_=sd)
        rrow = prows.tile([1, 512], BF16, tag="rr", name=f"r2row{n}")
        nc.scalar.copy(out=rrow, in_=r_f)
        rmurow = prows.tile([1, 512], BF16, tag="rmr", name=f"rmu2row{n}")
        nc.gpsimd.tensor_mul(rmurow, mu2, rrow)
        bp1 = ppbc2.tile([P, 512], F32, tag="bc", name=f"b2p1{n}")
        nc.tensor.matmul(bp1, onesK1, rrow)
        nc.scalar.copy(out=r2B[:, sl], in_=bp1)
        bp2 = ppbc2.tile([P, 512], F32, tag="bc", name=f"b2p2{n}")
        nc.tensor.matmul(bp2, onesK1, rmurow)
        nc.scalar.copy(out=mur2B[:, sl], in_=bp2)
        for c in range(DC):
            eng = nc.gpsimd if c % 4 == 1 else nc.vector
            eng.tensor_mul(cen[:, c, sl], h_t[:, c, sl], r2B[:, sl])
            eng.tensor_sub(cen[:, c, sl], cen[:, c, sl], mur2B[:, sl])
            nc.scalar.activation(out=g_t[:, c, sl], in_=cen[:, c, sl],
                                 func=AF.Gelu, bias=bln2_s[:, c:c + 1],
                                 scale=g2_s[:, c:c + 1])

    ppbc2.release()
    ppst2.release()
    ppE.release()

    # ============ FFN1 + lagged early FFN2 chains for 2 of the j's =====
    NJE = 2
    NKK = FC // 2
    poutc = tc.alloc_tile_pool(name="poutc", bufs=3)
    ppG = tc.alloc_tile_pool(name="ppG", bufs=3, space="PSUM")
    ppGe = tc.alloc_tile_pool(name="ppGe", bufs=NJE, space="PSUM")

    psf = {}

    def f1_chain(j, n):
        if n == 0:
            psf[j] = ppG.tile([P, Tq], F32, tag="mm", name=f"psf{j}")
        for k in range(NK1):
            kr = k % (DC // 2)
            nc.tensor.matmul(psf[j][:, ts(n, 512)],
                             w1_tiles[j][:, 2 * k:2 * k + 2, :],
                             g_t[:, 2 * kr:2 * kr + 2, ts(n, 512)],
                             start=(k == 0), stop=(k == NK1 - 1),
                             perf_mode=DR)
        if n == 1:
            nc.scalar.activation(out=f1g[:, j, :], in_=psf.pop(j),
                                 func=AF.Gelu, bias=b1_s[:, j:j + 1],
                                 scale=1.0 / W1_SCALE)

    def pso_step(psj, j, k, n, start, stop):
        kr = k % (FC // 2)
        nc.tensor.matmul(psj, w2_tiles[j][:, 2 * k:2 * k + 2, :],
                         f1g[:, 2 * kr:2 * kr + 2, ts(n, 512)],
                         start=start, stop=stop, perf_mode=DR)

    def outc_store(psj, j, n):
        outc = poutc.tile([P, 512], F32, tag="oc", name=f"outc{j}_{n}")
        nc.vector.scalar_tensor_tensor(
            out=outc, in0=psj, scalar=inv_w2_s,
            in1=h_t[:, j, ts(n, 512)], op0=ALU.mult, op1=ALU.add)
        if nzb2:
            nc.vector.tensor_scalar_add(outc, outc, b2_s[:, j:j + 1])
        nc.sync.dma_start(out=io["out"][j][:, ts(n, 512)], in_=outc)

    psoE = [ppGe.tile([P, 512], F32, tag="me", name=f"psoE{je}")
            for je in range(NJE)]
    nxt = [0]

    def emit_steps(limit_kk):
        while nxt[0] < min(limit_kk, NKK):
            kk = nxt[0]
            last = kk == NKK - 1
            for je in range(NJE):
                pso_step(psoE[je], je, kk, 0, start=(kk == 0),
                         stop=(last and not W2_COMP))
            if W2_COMP:
                for je in range(NJE):
                    pso_step(psoE[je], je, NKK + kk, 0, start=False,
                             stop=last)
            nxt[0] += 1

    LAG = 6
    LEAD = 2
    for j in range(LEAD):
        f1_chain(j, 0)
    for j in range(LEAD, FC + LEAD):
        if j < FC:
            f1_chain(j, 0)
        f1_chain(j - LEAD, 1)
        emit_steps((j - LEAD - LAG) // 2)
    emit_steps(NKK)
    for je in range(NJE):
        outc_store(psoE[je], je, 0)
    ppGe.release()
    ppG.release()

    # ============ FFN2 remainder + residual + store ============
    ppG2 = tc.alloc_tile_pool(name="ppG2", bufs=4, space="PSUM")
    for n in range(NQ):
        for j in range(DC):
            if n == 0 and j < NJE:
                continue
            psj = ppG2.tile([P, 512], F32, tag="mo", name=f"pso{j}_{n}")
            for k in range(NK2):
                pso_step(psj, j, k, n, start=(k == 0), stop=(k == NK2 - 1))
            outc_store(psj, j, n)

    ppG2.release()
    poutc.release()
    pw2.release()
    pw1.release()
    pf1.release()
    pwproj.release()
    pxq.release()
    pr2.release()
    pg.release()
    ph.release()
    prc.release()
    prows.release()
    consts.release()


# ----------------------------------------------------------------------------
# host side
# ----------------------------------------------------------------------------

def _stripe(v):
    """[n*P] -> [P, n] per-partition striping (feature f = c*P + p)."""
    v = np.asarray(v, np.float32)
    return np.ascontiguousarray(v.reshape(-1, P).T)


def _lhsT_stream8(W, scale):
    """fp8e4m3 lhsT stream of W*scale (scale undone on-device)."""
    din, dout = W.shape
    r = (W * scale).astype(f8e4).reshape(din // P, P, dout // P, P)
    return np.ascontiguousarray(r.transpose(2, 1, 0, 3))


def _lhsT_stream8c(W, scale):
    """Compensated fp8 lhsT stream: [Dout/P, P, 2*Din/P, P] with hi rounds
    first, then the fp8-quantized residuals at the same scale (residuals land
    in e4m3's subnormal range whose fixed absolute step makes W-quant error
    negligible). Device accumulates both chains in one PSUM group."""
    din, dout = W.shape
    r = (np.asarray(W, np.float32) * scale)
    hi = r.astype(f8e4)
    lo = (r - hi.astype(np.float32)).astype(f8e4)
    full = np.concatenate([hi.reshape(din // P, P, dout // P, P),
                           lo.reshape(din // P, P, dout // P, P)], axis=0)
    return np.ascontiguousarray(full.transpose(2, 1, 0, 3))


def prep_shared(inputs):
    f32 = np.float32
    g1 = np.asarray(inputs["ln1_g"], f32)
    b1n = np.asarray(inputs["ln1_b"], f32)
    W_ap = np.asarray(inputs["W_ap"], f32)
    b_ap = np.asarray(inputs["b_ap"], f32)
    W_qkv = np.asarray(inputs["W_qkv"], f32)
    b_qkv = np.asarray(inputs["b_qkv"], f32)
    W_proj = np.asarray(inputs["W_proj"], f32)

    # fold LN1 gamma and the whole attn pre-projection into W_qkv:
    # qkv = ln1(x) @ W_ap' @ W_qkv + (b_ap' @ W_qkv + b_qkv)
    W_eff = (g1[:, None] * W_ap) @ W_qkv
    b_eff = (b_ap + b1n @ W_ap) @ W_qkv + b_qkv
    shared = {
        "wq": _lhsT_stream8(W_eff[:, 0:D], WQ_SCALE),
        "wk": _lhsT_stream8(W_eff[:, D:2 * D], WQ_SCALE),
        "wv": np.ascontiguousarray(
            (W_eff[:, 2 * D:] * WQ_SCALE).astype(f8e4).reshape(DC, P, D)),
        "bqkv": _stripe(b_eff[:2 * D]),
        "bv": np.ascontiguousarray(np.asarray(b_eff[2 * D:], f32)),
        "wproj": _lhsT_stream8(W_proj, WQ_SCALE),
        "bproj": _stripe(np.asarray(inputs["b_proj"], f32)),
        "w1": (_lhsT_stream8c if W1_COMP else _lhsT_stream8)(
            np.asarray(inputs["W1"], f32), W1_SCALE),
        "b1": _stripe(np.asarray(inputs["b1"], f32)),
        "w2": (_lhsT_stream8c if W2_COMP else _lhsT_stream8)(
            np.asarray(inputs["W2"], f32), W2_SCALE),
        "b2": _stripe(np.asarray(inputs["b2"], f32)),
        "g2": _stripe(np.asarray(inputs["ln2_g"], f32)),
        "bln2": _stripe(np.asarray(inputs["ln2_b"], f32)),
    }
    return shared


def nz_flags(inputs):
    f32 = np.float32
    g1 = np.asarray(inputs["ln1_g"], f32)
    b1n = np.asarray(inputs["ln1_b"], f32)
    W_ap = np.asarray(inputs["W_ap"], f32)
    b_ap = np.asarray(inputs["b_ap"], f32)
    W_qkv = np.asarray(inputs["W_qkv"], f32)
    b_eff = (b_ap + b1n @ W_ap) @ W_qkv + np.asarray(inputs["b_qkv"], f32)
    return dict(
        nzqkv=bool(np.any(b_eff[:2 * D])),
        nzproj=bool(np.any(np.asarray(inputs["b_proj"]))),
        nzb2=bool(np.any(np.asarray(inputs["b2"]))),
    )


def prep_core_x(x, core):
    b, qh = core // 2, core % 2
    xTb = np.asarray(x[b], np.float32).T  # [D, T] view
    if qh:
        xTb = np.concatenate([xTb[:, Tq:], xTb[:, :Tq]], axis=1)
    return {
        "xt": np.ascontiguousarray(xTb.astype(f8e4).reshape(DC, P, T)),
        "xtq": np.ascontiguousarray(
            xTb[:, :Tq].astype(bf16).reshape(DC, P, Tq)),
    }


def assemble_output(results, dtype):
    out = np.empty((B, T, D), dtype)
    for c in range(N_CORES):
        b, qh = c // 2, c % 2
        arr = np.asarray(results[c]["out"]).reshape(D, Tq)
        out[b, qh * Tq:(qh + 1) * Tq, :] = arr.T
    return out


def kernel(**inputs):
    x = np.asarray(inputs["x"], np.float32)
    shared = prep_shared(inputs)
    nc = build_nc(**nz_flags(inputs))
    in_maps = [dict(shared, **prep_core_x(x, c)) for c in range(N_CORES)]
    res = run_bass_kernel_spmd(nc, in_maps, list(range(N_CORES)))
    return assemble_output(res.results, np.float32)


if __name__ == "__main__":
    nc = build_nc()
    print("built ok")
